# revision 1
# baseline (speedup 1.0000x reference)
"""BERT self-attention on 8 Trainium2 NeuronCores.

Sharding: data-parallel over batch (B=8 -> one batch element per core).
Each core computes full self-attention for its batch element:
  Q/K/V projections, per-head softmax(Q K^T / 8 + mask) V, output proj.

Layout strategy (per core):
  - Host passes xT = x.T [768,1024] and W.T [768,768] so every matmul
    contracts over the partition axis.
  - QT,KT [d, L] and V [L, d] are produced directly by the projections.
  - Attention runs transposed: ST[k,q] = K Q^T per head, so softmax's
    reduction axis (k) lands on partitions: exp via ScalarE with the
    attention mask as per-partition bias (no max subtraction: scores are
    ~N(0,1), |s|<~7, exp is safe in fp32); the denominator comes from a
    ones column appended to V (out row 64); P^T V accumulates ctx^T
    [d, q] which feeds the output projection as lhsT directly.
  - Matmul inputs are float32r (full PE speed at N>=512); Q^T/K^T are
    bf16 with K^T zero-padded per head to a full 128-row contraction
    (K=64 matmuls stream at half rate on TRN2). 1/denom is computed as
    exp(-ln d) on ScalarE (same ACT table set as the softmax exp).
    Accumulation and softmax run in fp32.
"""

import numpy as np

import concourse.bass as bass  # noqa: F401
import concourse.mybir as mybir
import concourse.tile as tile
from concourse import bacc
from concourse.bass_interp import get_hw_module
from concourse.bass_utils import run_bass_kernel_spmd

B, L, H = 8, 1024, 768
NH, HD = 12, 64
NC = H // 128          # 6 chunks of hidden dim
LC = L // 128          # 8 chunks of sequence dim
F32 = mybir.dt.float32
EXP = mybir.ActivationFunctionType.Exp


def build_bass(compute_rounded: bool = True):
    CDT = mybir.dt.float32r if compute_rounded else F32

    nc = bacc.Bacc("TRN2", debug=False, num_devices=8)

    # The kernel uses Exp (softmax) and Ln (reciprocal via exp(-ln d)).
    # Walrus's table-load pass would alternate exp_and_others /
    # natural_log sets (a ~1.3us ACT table DMA per switch, dozens per
    # kernel). Make the combined natural_log_exp_and_others set the only
    # provider of Exp/Ln so a single table load serves the whole kernel.
    from concourse.hw_specs import get_activation_tables

    _tabs = get_activation_tables(nc.m.arch)
    _E = mybir.ActivationFunctionType.Exp
    _L = mybir.ActivationFunctionType.Ln
    if "natural_log_exp_and_others" in _tabs:
        for _name, _fns in _tabs.items():
            if _name != "natural_log_exp_and_others":
                _fns.discard(_E)
                _fns.discard(_L)

    xt_e = nc.declare_dram_parameter("xt", [H, L], CDT, isOutput=False)
    wqt_e = nc.declare_dram_parameter("wqt", [H, H], CDT, isOutput=False)
    wkt_e = nc.declare_dram_parameter("wkt", [H, H], CDT, isOutput=False)
    wvt_e = nc.declare_dram_parameter("wvt", [H, H], CDT, isOutput=False)
    wot_e = nc.declare_dram_parameter("wot", [H, H], CDT, isOutput=False)
    bq_e = nc.declare_dram_parameter("bq", [H], F32, isOutput=False)
    bk_e = nc.declare_dram_parameter("bk", [H], F32, isOutput=False)
    bv_e = nc.declare_dram_parameter("bv", [H], CDT, isOutput=False)
    bo_e = nc.declare_dram_parameter("bo", [H], CDT, isOutput=False)
    mask_e = nc.declare_dram_parameter("mask", [L], F32, isOutput=False)
    out_e = nc.declare_dram_parameter("out", [L, H], F32, isOutput=True)

    with tile.TileContext(nc) as tc:
        with (
            tc.tile_pool(name="small", bufs=1) as small,
            tc.tile_pool(name="acts", bufs=1) as acts,
            tc.tile_pool(name="outp", bufs=2) as out_pool,
            tc.tile_pool(name="psA", bufs=2, space="PSUM") as psA,
            tc.tile_pool(name="psB", bufs=2, space="PSUM") as psB,
        ):
            # ---- constants / small tensors ----
            mask_sb = small.tile([128, LC], F32)
            nc.sync.dma_start(mask_sb[:], mask_e[:].rearrange("(c p) -> p c", p=128))
            bq_sb = small.tile([128, NC], F32)
            nc.sync.dma_start(bq_sb[:], bq_e[:].rearrange("(c p) -> p c", p=128))
            bk_sb = small.tile([128, NC], F32)
            nc.sync.dma_start(bk_sb[:], bk_e[:].rearrange("(c p) -> p c", p=128))
            bv_sb = small.tile([1, H], CDT)
            nc.sync.dma_start(bv_sb[:], bv_e[None, :])
            bo_sb = small.tile([1, H], CDT)
            nc.sync.dma_start(bo_sb[:], bo_e[None, :])
            ones32 = small.tile([128, 128], F32)
            nc.vector.memset(ones32[:], 1.0)
            ones = small.tile([128, 128], CDT)
            nc.vector.tensor_copy(ones[:], ones32[:])

            BF = mybir.dt.bfloat16
            qt_sb = acts.tile([128, NC, L], BF)
            kt_sb = acts.tile([128, NH, L], BF)  # per-head K^T, other 64 rows zero
            nc.gpsimd.memset(kt_sb[:], 0.0)
            v_sb = acts.tile([128, LC, NH, HD + 1], CDT)  # [..., 64] = ones col
            ctxt_sb = acts.tile([128, NC, L], CDT)

            nc.vector.tensor_copy(
                v_sb[:, :, :, HD],
                ones32[:, 0 : LC * NH].rearrange("p (a b) -> p a b", a=LC),
            )

            # =========== projection phase (xt + wv/wk/wq scoped) ===========
            with (
                tc.tile_pool(name="xt", bufs=1) as xt_pool,
                tc.tile_pool(name="w1", bufs=2) as w1,
            ):
                # interleave xt/wv chunk DMAs: V-proj's first matmuls only
                # need chunk 0 of each, so compute starts ~3us in instead of
                # waiting for the full 5.25MB load
                xt_sb = xt_pool.tile([128, NC, L], CDT)
                wv_sb = w1.tile([128, NC, H], CDT, tag="w")
                for c in range(NC):
                    nc.sync.dma_start(
                        wv_sb[:, c, :],
                        wvt_e[:].rearrange("(c p) d -> p c d", p=128)[:, c, :],
                    )
                    nc.sync.dma_start(
                        xt_sb[:, c, :],
                        xt_e[:].rearrange("(c p) q -> p c q", p=128)[:, c, :],
                    )
                for lc in range(LC):
                    ps = psA.tile([128, 1024], F32, tag="psA")
                    for off, width in ((0, 512), (512, 256)):
                        for kc in range(NC):
                            nc.tensor.matmul(
                                ps[:, off : off + width],
                                xt_sb[:, kc, lc * 128 : lc * 128 + 128],
                                wv_sb[:, kc, off : off + width],
                                start=(kc == 0),
                                stop=False,
                            )
                        nc.tensor.matmul(  # + bv (ones row x bias row)
                            ps[:, off : off + width],
                            ones[0:1, 0:128],
                            bv_sb[0:1, off : off + width],
                            start=False,
                            stop=True,
                        )
                    nc.vector.tensor_copy(
                        v_sb[:, lc, :, 0:HD],
                        ps[:, 0:H].rearrange("p (h d) -> p h d", d=HD),
                    )

                # ---- K^T then Q^T projections: out[d, q] = W x^T + b
                for w_e, b_sb, dst in ((wkt_e, bk_sb, kt_sb), (wqt_e, bq_sb, qt_sb)):
                    w_sb = w1.tile([128, NC, H], CDT, tag="w")
                    for c in range(NC):
                        nc.sync.dma_start(
                            w_sb[:, c, :],
                            w_e[:].rearrange("(c p) d -> p c d", p=128)[:, c, :],
                        )
                    for dc in range(NC):
                        ps = psA.tile([128, 1024], F32, tag="psA")
                        for qh in range(2):
                            o = qh * 512
                            for kc in range(NC):
                                nc.tensor.matmul(
                                    ps[:, o : o + 512],
                                    w_sb[:, kc, dc * 128 : dc * 128 + 128],
                                    xt_sb[:, kc, o : o + 512],
                                    start=(kc == 0),
                                    stop=(kc == NC - 1),
                                )
                        if dst is qt_sb:
                            nc.vector.tensor_scalar_add(
                                dst[:, dc, :], ps[:, :], b_sb[:, dc : dc + 1]
                            )
                        else:
                            nc.vector.tensor_scalar_add(
                                kt_sb[0:64, 2 * dc, :], ps[0:64, :],
                                b_sb[0:64, dc : dc + 1],
                            )
                            nc.vector.tensor_scalar_add(
                                kt_sb[64:128, 2 * dc + 1, :], ps[64:128, :],
                                b_sb[64:128, dc : dc + 1],
                            )

            # =========== attention + output projection ===========
            with (
                tc.tile_pool(name="w2", bufs=1) as w2,
                tc.tile_pool(name="et", bufs=6) as et_pool,
                tc.tile_pool(name="norm", bufs=1) as norm_pool,
            ):
                wo_sb = w2.tile([128, NC, H], CDT)
                for c in range(NC):
                    nc.sync.dma_start(
                        wo_sb[:, c, :],
                        wot_e[:].rearrange("(c p) d -> p c d", p=128)[:, c, :],
                    )

                def make_normalize(hp, ctxu_a, ctxu_b, ra, rb):
                    def emit():
                        # broadcast 1/denom over 64 partitions via f32r matmul,
                        # then scale ctx^T and store to ctxt_sb
                        for recip, ctxu, btag in ((ra, ctxu_a, "bca"),
                                                  (rb, ctxu_b, "bcb")):
                            bc = psA.tile([64, 1024], F32, tag="psA")
                            for o in (0, 512):
                                nc.tensor.matmul(
                                    bc[:, o : o + 512],
                                    ones[64:65, 0:64],
                                    recip[64:65, o : o + 512],
                                    start=True,
                                    stop=True,
                                )
                            bc_sb = norm_pool.tile([64, 1024], F32, tag=btag)
                            nc.vector.tensor_copy(bc_sb[:], bc[:])
                            if btag == "bca":
                                nc.gpsimd.tensor_tensor(
                                    ctxt_sb[0:64, hp, :], ctxu[0:64, :], bc_sb[:],
                                    mybir.AluOpType.mult,
                                )
                            else:
                                tmp_o = norm_pool.tile([64, 1024], CDT, tag="tmp")
                                nc.gpsimd.tensor_tensor(
                                    tmp_o[:], ctxu[0:64, :], bc_sb[:],
                                    mybir.AluOpType.mult,
                                )
                                # lift odd head to partitions 64:128 (DMA can
                                # cross partitions; DVE cannot)
                                nc.sync.dma_start(ctxt_sb[64:128, hp, :], tmp_o[:])
                    return emit

                pending = None
                pend_recips = None
                for hp in range(NH // 2):
                    ha, hb = 2 * hp, 2 * hp + 1
                    # per head: rows 0:64 = ctx^T, row 64 = softmax denominator
                    ctx_a = psB.tile([128, 1024], F32, tag="psB")
                    ctx_b = psB.tile([128, 1024], F32, tag="psB")

                    def emit_pv(kc, et_a, et_b, ctx_a=ctx_a, ctx_b=ctx_b,
                                ha=ha, hb=hb):
                        first, last = kc == 0, kc == LC - 1
                        for qh in range(2):
                            o = qh * 512
                            # ctx^T[d, q] += V^T P^T ; row 64 = denominator
                            nc.tensor.matmul(
                                ctx_a[0 : HD + 1, o : o + 512],
                                v_sb[:, kc, ha, :],
                                et_a[:, o : o + 512],
                                start=first,
                                stop=last,
                            )
                            nc.tensor.matmul(
                                ctx_b[0 : HD + 1, o : o + 512],
                                v_sb[:, kc, hb, :],
                                et_b[:, o : o + 512],
                                start=first,
                                stop=last,
                            )

                    pv_q = []  # software pipeline: PV(kc-1) after ST(kc)
                    for kc in range(LC):
                        st_a = psA.tile([128, 1024], F32, tag="psA")
                        st_b = psA.tile([128, 1024], F32, tag="psA")
                        for qh in range(2):
                            o = qh * 512
                            # S^T[k, q] = K Q^T for both heads (row-group packed)
                            nc.tensor.matmul(
                                st_a[:, o : o + 512],
                                kt_sb[:, ha, kc * 128 : kc * 128 + 128],
                                qt_sb[:, hp, o : o + 512],
                                start=True,
                                stop=True,
                            )
                            nc.tensor.matmul(
                                st_b[:, o : o + 512],
                                kt_sb[:, hb, kc * 128 : kc * 128 + 128],
                                qt_sb[:, hp, o : o + 512],
                                start=True,
                                stop=True,
                            )
                        # P^T = exp(S^T/8 + mask_k)
                        et_a = et_pool.tile([128, 1024], CDT, tag="et")
                        et_b = et_pool.tile([128, 1024], CDT, tag="et")
                        nc.scalar.activation(
                            et_a[:], st_a[:], EXP,
                            bias=mask_sb[:, kc : kc + 1], scale=0.125,
                        )
                        nc.scalar.activation(
                            et_b[:], st_b[:], EXP,
                            bias=mask_sb[:, kc : kc + 1], scale=0.125,
                        )
                        pv_q.append((kc, et_a, et_b))
                        if kc >= 1:
                            emit_pv(*pv_q.pop(0))
                        if kc == 1 and pend_recips is not None:
                            # previous pair's head-b ACT recip: emitted after
                            # this pair's first exps so it doesn't delay them
                            # in the ScalarE FIFO (st-slot recycling couples
                            # those exps to the PE's ST stream)
                            pend_recips()
                            pend_recips = None
                        if kc == 5 and pending is not None:
                            # previous pair's normalize, emitted here so its
                            # matmuls never head-of-line-block the PE
                            pending()
                            pending = None
                    emit_pv(*pv_q.pop(0))
                    # evacuate ctx+denominator to SBUF immediately: frees the
                    # PSUM slot and takes the reciprocal off the PE path
                    ctxu_a = norm_pool.tile([65, 1024], F32, tag="cua")
                    nc.vector.tensor_copy(ctxu_a[:], ctx_a[0:65, :])
                    ctxu_b = norm_pool.tile([65, 1024], F32, tag="cub")
                    nc.vector.tensor_copy(ctxu_b[:], ctx_b[0:65, :])
                    # 1/d = exp(-ln d) on ScalarE: Log+Exp share one ACT
                    # table set, ~1e-5 rel err, and it keeps the reciprocal
                    # off the (slow, 1-lane) DVE path entirely
                    LOG = mybir.ActivationFunctionType.Ln
                    ra = norm_pool.tile([65, 1024], CDT, tag="ra")
                    rb = norm_pool.tile([65, 1024], CDT, tag="rb")
                    if hp == NH // 2 - 1:
                        # last pair: no following matmul stream hides the DVE
                        # reciprocal's ~8us latency; use the short ACT path
                        lna = norm_pool.tile([65, 1024], F32, tag="lna")
                        nc.scalar.activation(lna[64:65, :], ctxu_a[64:65, :], LOG)
                        nc.scalar.activation(
                            ra[64:65, :], lna[64:65, :], EXP, scale=-1.0
                        )
                    else:
                        ra32 = norm_pool.tile([65, 1024], F32, tag="ra32")
                        nc.vector.reciprocal(ra32[64:65, :], ctxu_a[64:65, :])
                        nc.vector.tensor_copy(ra[64:65, :], ra32[64:65, :])

                    def emit_recip_b(ctxu_b=ctxu_b, rb=rb):
                        lnb = norm_pool.tile([65, 1024], F32, tag="lnb")
                        nc.scalar.activation(lnb[64:65, :], ctxu_b[64:65, :], LOG)
                        nc.scalar.activation(
                            rb[64:65, :], lnb[64:65, :], EXP, scale=-1.0
                        )

                    if hp == NH // 2 - 1:
                        emit_recip_b()
                    else:
                        pend_recips = emit_recip_b
                    pending = make_normalize(hp, ctxu_a, ctxu_b, ra, rb)
                if pend_recips is not None:
                    pend_recips()
                    pend_recips = None
                if pending is not None:
                    pending()
                    pending = None

                # ---- output projection: out[q, o] = ctx Wo^T + bo ----
                for lc in range(LC):
                    ps = psA.tile([128, 1024], F32, tag="psA")
                    for off, width in ((0, 512), (512, 256)):
                        for c in range(NC):
                            nc.tensor.matmul(
                                ps[:, off : off + width],
                                ctxt_sb[:, c, lc * 128 : lc * 128 + 128],
                                wo_sb[:, c, off : off + width],
                                start=(c == 0),
                                stop=False,
                            )
                        nc.tensor.matmul(  # + bo
                            ps[:, off : off + width],
                            ones[0:1, 0:128],
                            bo_sb[0:1, off : off + width],
                            start=False,
                            stop=True,
                        )
                    o_sb = out_pool.tile([128, H], F32, tag="outp")
                    nc.vector.tensor_copy(o_sb[:], ps[:, 0:H])
                    nc.sync.dma_start(out_e[lc * 128 : lc * 128 + 128, :], o_sb[:])

    nc.finalize()
    nc.m = get_hw_module(nc.m)
    return nc


_NC_CACHE = {}


def _get_nc(compute_rounded: bool = True):
    if compute_rounded not in _NC_CACHE:
        _NC_CACHE[compute_rounded] = build_bass(compute_rounded)
    return _NC_CACHE[compute_rounded]


def make_in_maps(inputs):
    f = lambda a: np.ascontiguousarray(np.asarray(a, dtype=np.float32))  # noqa: E731
    hs = f(inputs["hidden_states"])
    mask = f(inputs["attention_mask"]).reshape(B, L)
    shared = {
        "wqt": f(np.asarray(inputs["Wq"]).T),
        "wkt": f(np.asarray(inputs["Wk"]).T),
        "wvt": f(np.asarray(inputs["Wv"]).T),
        "wot": f(np.asarray(inputs["Wo"]).T),
        "bq": f(inputs["bq"]),
        "bk": f(inputs["bk"]),
        "bv": f(inputs["bv"]),
        "bo": f(inputs["bo"]),
    }
    return [
        {"xt": f(hs[b].T), "mask": mask[b], **shared}
        for b in range(B)
    ]


def run_spmd(inputs, trace=False, compute_rounded=True):
    nc = _get_nc(compute_rounded)
    res = run_bass_kernel_spmd(nc, make_in_maps(inputs), list(range(B)), trace=trace)
    out = np.stack([res.results[b]["out"] for b in range(B)]).astype(np.float32)
    return out, res


def kernel(**inputs) -> np.ndarray:
    out, _ = run_spmd(inputs, trace=False)
    return out



# revision 3
# speedup vs baseline: 1.0355x; 1.0355x over previous
"""BERT self-attention on 8 Trainium2 NeuronCores.

Sharding: data-parallel over batch (B=8 -> one batch element per core).
Each core computes full self-attention for its batch element:
  Q/K/V projections, per-head softmax(Q K^T / 8 + mask) V, output proj.

Layout strategy (per core):
  - Host passes xt = x.T [768,1024] and W.T [768,768] in bf16 so every
    matmul contracts over the partition axis and streams at 1 cyc/row.
  - QT,KT [d, L] and V [L, d] are produced directly by the projections.
  - Attention runs transposed: ST[k,q] = K Q^T per head, so softmax's
    reduction axis (k) lands on partitions: exp via ScalarE with the
    attention mask as per-partition bias (no max subtraction: scores are
    ~N(0,1), |s|<~7, exp is safe in fp32); the denominator comes from a
    ones column appended to V (out row 64); P^T V accumulates ctx^T
    [d, q] which feeds the output projection as lhsT directly.
  - All matmul inputs are bf16 (fp32 PSUM accumulation); measured rel
    err vs the fp32 reference is ~7e-3.
  - The PE does only the structural matmuls: QKV/out biases are folded
    into the PSUM->SBUF evacuation ops against DMA-broadcast bias rows,
    and the softmax 1/denom broadcast is computed entirely off the PE:
    the two denominator rows are DMA-packed across 128 partitions
    ([128,16]), reciprocated in one cheap DVE op, DMA-unpacked, and
    partition-broadcast on GpSimd.  This keeps ScalarE exp-only, which
    is what paces the attention inner loop.
  - Startup: weight chunks load on the Sync HWDGE ring while xt chunks
    load on the Scalar HWDGE ring, halving time-to-first-matmul.
"""

import numpy as np
import ml_dtypes

import concourse.bass as bass  # noqa: F401
import concourse.mybir as mybir
import concourse.tile as tile
from concourse import bacc
from concourse.bass_interp import get_hw_module
from concourse.bass_utils import run_bass_kernel_spmd

B, L, H = 8, 1024, 768
NH, HD = 12, 64
NC = H // 128          # 6 chunks of hidden dim
LC = L // 128          # 8 chunks of sequence dim
F32 = mybir.dt.float32
BF = mybir.dt.bfloat16
EXP = mybir.ActivationFunctionType.Exp


def build_bass(compute_rounded: bool = True):
    del compute_rounded  # single all-bf16 variant

    nc = bacc.Bacc("TRN2", debug=False, num_devices=8)

    xt_e = nc.declare_dram_parameter("xt", [H, L], BF, isOutput=False)
    wqt_e = nc.declare_dram_parameter("wqt", [H, H], BF, isOutput=False)
    wkt_e = nc.declare_dram_parameter("wkt", [H, H], BF, isOutput=False)
    wvt_e = nc.declare_dram_parameter("wvt", [H, H], BF, isOutput=False)
    wot_e = nc.declare_dram_parameter("wot", [H, H], BF, isOutput=False)
    bq_e = nc.declare_dram_parameter("bq", [H], F32, isOutput=False)
    bk_e = nc.declare_dram_parameter("bk", [H], F32, isOutput=False)
    bv_e = nc.declare_dram_parameter("bv", [H], F32, isOutput=False)
    bo_e = nc.declare_dram_parameter("bo", [H], F32, isOutput=False)
    mask_e = nc.declare_dram_parameter("mask", [L], F32, isOutput=False)
    out_e = nc.declare_dram_parameter("out", [L, H], F32, isOutput=True)

    with tile.TileContext(nc) as tc:
        with (
            tc.tile_pool(name="small", bufs=1) as small,
            tc.tile_pool(name="acts", bufs=1) as acts,
            tc.tile_pool(name="outp", bufs=2) as out_pool,
            tc.tile_pool(name="psA", bufs=2, space="PSUM") as psA,
            tc.tile_pool(name="psB", bufs=2, space="PSUM") as psB,
        ):
            # ---- constants / small tensors ----
            mask_sb = small.tile([128, LC], F32)
            nc.sync.dma_start(mask_sb[:], mask_e[:].rearrange("(c p) -> p c", p=128))
            bq_sb = small.tile([128, NC], F32)
            nc.sync.dma_start(bq_sb[:], bq_e[:].rearrange("(c p) -> p c", p=128))
            bk_sb = small.tile([128, NC], F32)
            nc.sync.dma_start(bk_sb[:], bk_e[:].rearrange("(c p) -> p c", p=128))
            # bias rows replicated across all partitions (free-dim biases)
            bv_sb = small.tile([128, H], F32)
            nc.sync.dma_start(bv_sb[:], bv_e[None, :].to_broadcast((128, H)))
            bo_sb = small.tile([128, H], F32)
            nc.sync.dma_start(bo_sb[:], bo_e[None, :].to_broadcast((128, H)))

            qt_sb = acts.tile([128, NC, L], BF)
            kt_sb = acts.tile([128, NH, L], BF)  # per-head K^T, other 64 rows zero
            nc.gpsimd.memset(kt_sb[:], 0.0)
            v_sb = acts.tile([128, LC, NH, HD + 1], BF)  # [..., 64] = ones col
            ctxt_sb = acts.tile([128, NC, L], BF)
            nc.vector.memset(v_sb[:, :, :, HD], 1.0)

            # =========== projection phase (xt + wv/wk/wq scoped) ===========
            with (
                tc.tile_pool(name="xt", bufs=1) as xt_pool,
                tc.tile_pool(name="w1", bufs=2) as w1,
            ):
                # weights ride the Sync HWDGE ring, xt the Scalar ring, so
                # the V-proj's first matmul only waits ~one chunk per ring
                xt_sb = xt_pool.tile([128, NC, L], BF)
                wv_sb = w1.tile([128, NC, H], BF, tag="w")
                for c in range(NC):
                    nc.sync.dma_start(
                        wv_sb[:, c, :],
                        wvt_e[:].rearrange("(c p) d -> p c d", p=128)[:, c, :],
                    )
                    nc.scalar.dma_start(
                        xt_sb[:, c, :],
                        xt_e[:].rearrange("(c p) q -> p c q", p=128)[:, c, :],
                    )
                for lc in range(LC):
                    ps = psA.tile([128, 1024], F32, tag="psA")
                    for off, width in ((0, 512), (512, 256)):
                        for kc in range(NC):
                            nc.tensor.matmul(
                                ps[:, off : off + width],
                                xt_sb[:, kc, lc * 128 : lc * 128 + 128],
                                wv_sb[:, kc, off : off + width],
                                start=(kc == 0),
                                stop=(kc == NC - 1),
                            )
                    # evacuate + bv add (bias varies along free dim)
                    nc.vector.tensor_tensor(
                        v_sb[:, lc, :, 0:HD],
                        ps[:, 0:H].rearrange("p (h d) -> p h d", d=HD),
                        bv_sb[:].rearrange("p (h d) -> p h d", d=HD),
                        mybir.AluOpType.add,
                    )

                # ---- K^T then Q^T projections: out[d, q] = W x^T + b
                for w_e, b_sb, dst in ((wkt_e, bk_sb, kt_sb), (wqt_e, bq_sb, qt_sb)):
                    w_sb = w1.tile([128, NC, H], BF, tag="w")
                    for c in range(NC):
                        nc.sync.dma_start(
                            w_sb[:, c, :],
                            w_e[:].rearrange("(c p) d -> p c d", p=128)[:, c, :],
                        )
                    for dc in range(NC):
                        ps = psA.tile([128, 1024], F32, tag="psA")
                        for qh in range(2):
                            o = qh * 512
                            for kc in range(NC):
                                nc.tensor.matmul(
                                    ps[:, o : o + 512],
                                    w_sb[:, kc, dc * 128 : dc * 128 + 128],
                                    xt_sb[:, kc, o : o + 512],
                                    start=(kc == 0),
                                    stop=(kc == NC - 1),
                                )
                        if dst is qt_sb:
                            nc.vector.tensor_scalar_add(
                                dst[:, dc, :], ps[:, :], b_sb[:, dc : dc + 1]
                            )
                        else:
                            nc.vector.tensor_scalar_add(
                                kt_sb[0:64, 2 * dc, :], ps[0:64, :],
                                b_sb[0:64, dc : dc + 1],
                            )
                            nc.vector.tensor_scalar_add(
                                kt_sb[64:128, 2 * dc + 1, :], ps[64:128, :],
                                b_sb[64:128, dc : dc + 1],
                            )

            # =========== attention + output projection ===========
            with (
                tc.tile_pool(name="w2", bufs=1) as w2,
                tc.tile_pool(name="et", bufs=6) as et_pool,
                tc.tile_pool(name="norm", bufs=1) as norm_pool,
            ):
                wo_sb = w2.tile([128, NC, H], BF)
                for c in range(NC):
                    nc.sync.dma_start(
                        wo_sb[:, c, :],
                        wot_e[:].rearrange("(c p) d -> p c d", p=128)[:, c, :],
                    )

                def emit_normalize(hp, ctxu_a, ctxu_b):
                    # softmax 1/denom, entirely off the PE/ACT engines:
                    # pack both denominator rows across 128 partitions
                    # (q = p*8+i), one cheap DVE reciprocal, unpack, then
                    # GpSimd partition-broadcast to 64 rows for the scale.
                    dpk = norm_pool.tile([128, 16], F32, tag="dpk")
                    nc.sync.dma_start(
                        dpk[:, 0:8],
                        ctxu_a[64:65, :].rearrange("o (p i) -> o p i", p=128),
                    )
                    nc.sync.dma_start(
                        dpk[:, 8:16],
                        ctxu_b[64:65, :].rearrange("o (p i) -> o p i", p=128),
                    )
                    rpk = norm_pool.tile([128, 16], F32, tag="rpk")
                    nc.vector.reciprocal(rpk[:], dpk[:])
                    rra = norm_pool.tile([1, 1024], F32, tag="rra")
                    nc.sync.dma_start(
                        rra[0:1, :].rearrange("o (p i) -> o p i", p=128),
                        rpk[:, 0:8],
                    )
                    rrb = norm_pool.tile([1, 1024], F32, tag="rrb")
                    nc.sync.dma_start(
                        rrb[0:1, :].rearrange("o (p i) -> o p i", p=128),
                        rpk[:, 8:16],
                    )
                    bca = norm_pool.tile([64, 1024], F32, tag="bca")
                    nc.gpsimd.partition_broadcast(bca[:], rra[0:1, :], channels=64)
                    bcb = norm_pool.tile([64, 1024], F32, tag="bcb")
                    nc.gpsimd.partition_broadcast(bcb[:], rrb[0:1, :], channels=64)
                    nc.vector.tensor_tensor(
                        ctxt_sb[0:64, hp, :], ctxu_a[0:64, :], bca[:],
                        mybir.AluOpType.mult,
                    )
                    tmp_o = norm_pool.tile([64, 1024], BF, tag="tmp")
                    nc.gpsimd.tensor_tensor(
                        tmp_o[:], ctxu_b[0:64, :], bcb[:],
                        mybir.AluOpType.mult,
                    )
                    # lift odd head to partitions 64:128 (DMA can cross
                    # partitions; DVE cannot)
                    nc.sync.dma_start(ctxt_sb[64:128, hp, :], tmp_o[:])

                for hp in range(NH // 2):
                    ha, hb = 2 * hp, 2 * hp + 1
                    # per head: rows 0:64 = ctx^T, row 64 = softmax denominator
                    ctx_a = psB.tile([128, 1024], F32, tag="psB")
                    ctx_b = psB.tile([128, 1024], F32, tag="psB")

                    def emit_pv(kc, et_a, et_b, ctx_a=ctx_a, ctx_b=ctx_b,
                                ha=ha, hb=hb):
                        first, last = kc == 0, kc == LC - 1
                        for qh in range(2):
                            o = qh * 512
                            # ctx^T[d, q] += V^T P^T ; row 64 = denominator
                            nc.tensor.matmul(
                                ctx_a[0 : HD + 1, o : o + 512],
                                v_sb[:, kc, ha, :],
                                et_a[:, o : o + 512],
                                start=first,
                                stop=last,
                            )
                            nc.tensor.matmul(
                                ctx_b[0 : HD + 1, o : o + 512],
                                v_sb[:, kc, hb, :],
                                et_b[:, o : o + 512],
                                start=first,
                                stop=last,
                            )

                    pv_q = []  # software pipeline: PV(kc-1) after ST(kc)
                    for kc in range(LC):
                        # per-head ST then its exp immediately, so ScalarE
                        # starts head a's exp while the PE streams head b
                        st_a = psA.tile([128, 1024], F32, tag="psA")
                        st_b = psA.tile([128, 1024], F32, tag="psA")
                        for o in (0, 512):
                            nc.tensor.matmul(
                                st_a[:, o : o + 512],
                                kt_sb[:, ha, kc * 128 : kc * 128 + 128],
                                qt_sb[:, hp, o : o + 512],
                                start=True,
                                stop=True,
                            )
                        et_a = et_pool.tile([128, 1024], BF, tag="et")
                        nc.scalar.activation(
                            et_a[:], st_a[:], EXP,
                            bias=mask_sb[:, kc : kc + 1], scale=0.125,
                        )
                        for o in (0, 512):
                            nc.tensor.matmul(
                                st_b[:, o : o + 512],
                                kt_sb[:, hb, kc * 128 : kc * 128 + 128],
                                qt_sb[:, hp, o : o + 512],
                                start=True,
                                stop=True,
                            )
                        et_b = et_pool.tile([128, 1024], BF, tag="et")
                        nc.scalar.activation(
                            et_b[:], st_b[:], EXP,
                            bias=mask_sb[:, kc : kc + 1], scale=0.125,
                        )
                        pv_q.append((kc, et_a, et_b))
                        if kc >= 1:
                            emit_pv(*pv_q.pop(0))
                    emit_pv(*pv_q.pop(0))
                    # evacuate ctx+denominator to SBUF immediately: frees the
                    # PSUM slot; normalize runs on DVE/Pool/DMA only and
                    # overlaps the next pair's matmul stream
                    ctxu_a = norm_pool.tile([65, 1024], F32, tag="cua")
                    nc.vector.tensor_copy(ctxu_a[:], ctx_a[0:65, :])
                    ctxu_b = norm_pool.tile([65, 1024], F32, tag="cub")
                    nc.vector.tensor_copy(ctxu_b[:], ctx_b[0:65, :])
                    emit_normalize(hp, ctxu_a, ctxu_b)

                # ---- output projection: out[q, o] = ctx Wo^T + bo ----
                for lc in range(LC):
                    ps = psA.tile([128, 1024], F32, tag="psA")
                    for off, width in ((0, 512), (512, 256)):
                        for c in range(NC):
                            nc.tensor.matmul(
                                ps[:, off : off + width],
                                ctxt_sb[:, c, lc * 128 : lc * 128 + 128],
                                wo_sb[:, c, off : off + width],
                                start=(c == 0),
                                stop=(c == NC - 1),
                            )
                    o_sb = out_pool.tile([128, H], F32, tag="outp")
                    nc.vector.tensor_tensor(
                        o_sb[:], ps[:, 0:H], bo_sb[:], mybir.AluOpType.add
                    )
                    nc.sync.dma_start(out_e[lc * 128 : lc * 128 + 128, :], o_sb[:])

    nc.finalize()
    nc.m = get_hw_module(nc.m)
    return nc


_NC_CACHE = {}


def _get_nc(compute_rounded: bool = True):
    if compute_rounded not in _NC_CACHE:
        _NC_CACHE[compute_rounded] = build_bass(compute_rounded)
    return _NC_CACHE[compute_rounded]


def make_in_maps(inputs):
    f = lambda a: np.ascontiguousarray(np.asarray(a, dtype=np.float32))  # noqa: E731
    fb = lambda a: np.ascontiguousarray(  # noqa: E731
        np.asarray(a, dtype=np.float32).astype(ml_dtypes.bfloat16)
    )
    hs = f(inputs["hidden_states"])
    mask = f(inputs["attention_mask"]).reshape(B, L)
    shared = {
        "wqt": fb(np.asarray(inputs["Wq"]).T),
        "wkt": fb(np.asarray(inputs["Wk"]).T),
        "wvt": fb(np.asarray(inputs["Wv"]).T),
        "wot": fb(np.asarray(inputs["Wo"]).T),
        "bq": f(inputs["bq"]),
        "bk": f(inputs["bk"]),
        "bv": f(inputs["bv"]),
        "bo": f(inputs["bo"]),
    }
    return [
        {"xt": fb(hs[b].T), "mask": mask[b], **shared}
        for b in range(B)
    ]


def run_spmd(inputs, trace=False, compute_rounded=True):
    nc = _get_nc(compute_rounded)
    res = run_bass_kernel_spmd(nc, make_in_maps(inputs), list(range(B)), trace=trace)
    out = np.stack([res.results[b]["out"] for b in range(B)]).astype(np.float32)
    return out, res


def kernel(**inputs) -> np.ndarray:
    out, _ = run_spmd(inputs, trace=False)
    return out


# revision 9
# speedup vs baseline: 1.0872x; 1.0499x over previous
"""BERT self-attention on 8 Trainium2 NeuronCores.

Sharding: data-parallel over batch (B=8 -> one batch element per core).
Each core computes full self-attention for its batch element:
  Q/K/V projections, per-head softmax(Q K^T / 8 + mask) V, output proj.

Layout strategy (per core):
  - Host passes xt = x.T [768,1024] and W.T [768,768] in bf16 so every
    matmul contracts over the partition axis and streams at 1 cyc/row.
  - QT,KT [d, L] and V [L, d] are produced directly by the projections.
  - Attention runs transposed: ST[k,q] = K Q^T per head, so softmax's
    reduction axis (k) lands on partitions: exp via ScalarE with the
    attention mask as per-partition bias (no max subtraction: scores are
    ~N(0,1), |s|<~7, exp is safe in fp32); the denominator comes from a
    ones column appended to V (out row 64); P^T V accumulates ctx^T
    [d, q] which feeds the output projection as lhsT directly.
  - All matmul inputs are bf16 (fp32 PSUM accumulation); measured rel
    err vs the fp32 reference is ~7e-3.
  - The PE does only the structural matmuls: QKV/out biases are folded
    into the PSUM->SBUF evacuation ops against DMA-broadcast bias rows,
    and the softmax 1/denom broadcast is computed entirely off the PE:
    the two denominator rows are DMA-packed across 128 partitions
    ([128,16]), reciprocated in one cheap DVE op, DMA-unpacked, and
    partition-broadcast on GpSimd.  This keeps ScalarE exp-only, which
    is what paces the attention inner loop.
  - Startup: weight chunks load on the Sync HWDGE ring while xt chunks
    load on the Scalar HWDGE ring, halving time-to-first-matmul.
"""

import numpy as np
import ml_dtypes

import concourse.bass as bass  # noqa: F401
import concourse.mybir as mybir
import concourse.tile as tile
from concourse import bacc
from concourse.bass_interp import get_hw_module
from concourse.bass_utils import run_bass_kernel_spmd

B, L, H = 8, 1024, 768
NH, HD = 12, 64
NC = H // 128          # 6 chunks of hidden dim
LC = L // 128          # 8 chunks of sequence dim
F32 = mybir.dt.float32
BF = mybir.dt.bfloat16
EXP = mybir.ActivationFunctionType.Exp


def build_bass(compute_rounded: bool = True):
    del compute_rounded  # single all-bf16 variant

    nc = bacc.Bacc("TRN2", debug=False, num_devices=8)

    xt_e = nc.declare_dram_parameter("xt", [H, L], BF, isOutput=False)
    wqt_e = nc.declare_dram_parameter("wqt", [H, H], BF, isOutput=False)
    wkt_e = nc.declare_dram_parameter("wkt", [H, H], BF, isOutput=False)
    wvt_e = nc.declare_dram_parameter("wvt", [H, H], BF, isOutput=False)
    wot_e = nc.declare_dram_parameter("wot", [H, H], BF, isOutput=False)
    bq_e = nc.declare_dram_parameter("bq", [H], F32, isOutput=False)
    bk_e = nc.declare_dram_parameter("bk", [H], F32, isOutput=False)
    bv_e = nc.declare_dram_parameter("bv", [H], F32, isOutput=False)
    bo_e = nc.declare_dram_parameter("bo", [H], F32, isOutput=False)
    mask_e = nc.declare_dram_parameter("mask", [L], F32, isOutput=False)
    out_e = nc.declare_dram_parameter("out", [L, H], F32, isOutput=True)
    # DRAM scratch for the softmax reciprocal broadcast roundtrip
    rscr_e = nc.dram_tensor("rscr", [2048], F32)

    with tile.TileContext(nc) as tc:
        with (
            tc.tile_pool(name="small", bufs=1) as small,
            tc.tile_pool(name="acts", bufs=1) as acts,
            tc.tile_pool(name="outp", bufs=2) as out_pool,
            tc.tile_pool(name="psA", bufs=2, space="PSUM") as psA,
            tc.tile_pool(name="psB", bufs=2, space="PSUM") as psB,
        ):
            # ---- constants / small tensors (DMAs issued after wv/xt: the
            # V-proj's first matmul gates on wv/xt chunk 0, these don't) ----
            mask_sb = small.tile([128, LC], F32)
            bq_sb = small.tile([128, NC], F32)
            bk_sb = small.tile([128, NC], F32)
            bv_sb = small.tile([128, H], F32)
            bo_sb = small.tile([128, H], F32)

            def load_smalls():
                nc.sync.dma_start(
                    mask_sb[:], mask_e[:].rearrange("(c p) -> p c", p=128)
                )
                nc.sync.dma_start(bq_sb[:], bq_e[:].rearrange("(c p) -> p c", p=128))
                nc.sync.dma_start(bk_sb[:], bk_e[:].rearrange("(c p) -> p c", p=128))
                # bias rows replicated across all partitions (free-dim biases)
                nc.sync.dma_start(bv_sb[:], bv_e[None, :].to_broadcast((128, H)))
                nc.sync.dma_start(bo_sb[:], bo_e[None, :].to_broadcast((128, H)))

            qt_sb = acts.tile([128, NC, L], BF)
            kt_sb = acts.tile([128, NH, L], BF)  # per-head K^T, other 64 rows zero
            nc.gpsimd.memset(kt_sb[:], 0.0)
            v_sb = acts.tile([128, LC, NH, HD + 1], BF)  # [..., 64] = ones col
            ctxt_sb = acts.tile([128, NC, L], BF)
            nc.vector.memset(v_sb[:, :, :, HD], 1.0)

            # =========== projection phase (xt + wv/wk/wq scoped) ===========
            with (
                tc.tile_pool(name="xt", bufs=1) as xt_pool,
                tc.tile_pool(name="w1", bufs=2) as w1,
            ):
                # weights ride the Sync HWDGE ring, xt the Scalar ring, so
                # the V-proj's first matmul only waits ~one chunk per ring
                xt_sb = xt_pool.tile([128, NC, L], BF)
                wv_sb = w1.tile([128, NC, H], BF, tag="w")
                for c in range(NC):
                    nc.sync.dma_start(
                        wv_sb[:, c, :],
                        wvt_e[:].rearrange("(c p) d -> p c d", p=128)[:, c, :],
                    )
                    nc.scalar.dma_start(
                        xt_sb[:, c, :],
                        xt_e[:].rearrange("(c p) q -> p c q", p=128)[:, c, :],
                    )
                load_smalls()
                for lc in range(LC):
                    ps = psA.tile([128, 1024], F32, tag="psA")
                    for off, width in ((0, 512), (512, 256)):
                        for kc in range(NC):
                            nc.tensor.matmul(
                                ps[:, off : off + width],
                                xt_sb[:, kc, lc * 128 : lc * 128 + 128],
                                wv_sb[:, kc, off : off + width],
                                start=(kc == 0),
                                stop=(kc == NC - 1),
                            )
                    # evacuate + bv add (bias varies along free dim)
                    nc.vector.tensor_tensor(
                        v_sb[:, lc, :, 0:HD],
                        ps[:, 0:H].rearrange("p (h d) -> p h d", d=HD),
                        bv_sb[:].rearrange("p (h d) -> p h d", d=HD),
                        mybir.AluOpType.add,
                    )

                # ---- K^T then Q^T projections: out[d, q] = W x^T + b
                for w_e, b_sb, dst in ((wkt_e, bk_sb, kt_sb), (wqt_e, bq_sb, qt_sb)):
                    w_sb = w1.tile([128, NC, H], BF, tag="w")
                    nc.sync.dma_start(
                        w_sb[:], w_e[:].rearrange("(c p) d -> p c d", p=128)
                    )
                    for dc in range(NC):
                        ps = psA.tile([128, 1024], F32, tag="psA")
                        for qh in range(2):
                            o = qh * 512
                            for kc in range(NC):
                                nc.tensor.matmul(
                                    ps[:, o : o + 512],
                                    w_sb[:, kc, dc * 128 : dc * 128 + 128],
                                    xt_sb[:, kc, o : o + 512],
                                    start=(kc == 0),
                                    stop=(kc == NC - 1),
                                )
                        if dst is qt_sb:
                            nc.vector.tensor_scalar_add(
                                dst[:, dc, :], ps[:, :], b_sb[:, dc : dc + 1]
                            )
                        else:
                            nc.vector.tensor_scalar_add(
                                kt_sb[0:64, 2 * dc, :], ps[0:64, :],
                                b_sb[0:64, dc : dc + 1],
                            )
                            nc.vector.tensor_scalar_add(
                                kt_sb[64:128, 2 * dc + 1, :], ps[64:128, :],
                                b_sb[64:128, dc : dc + 1],
                            )

            # =========== attention + output projection ===========
            with (
                tc.tile_pool(name="w2", bufs=1) as w2,
                tc.tile_pool(name="et", bufs=6) as et_pool,
                tc.tile_pool(name="norm", bufs=1) as norm_pool,
            ):
                wo_sb = w2.tile([128, NC, H], BF)
                nc.sync.dma_start(
                    wo_sb[:], wot_e[:].rearrange("(c p) d -> p c d", p=128)
                )

                rscr_pi = rscr_e[:].rearrange("(p i) -> p i", p=128)

                def emit_normalize(hp, ctxu_a, ctxu_b):
                    # softmax 1/denom, entirely off the PE/ACT engines:
                    # pack both denominator rows across 128 partitions
                    # (q = p*8+i), one cheap DVE reciprocal, then a DRAM
                    # roundtrip to replicate the reciprocals to 64 rows
                    # (engines can't broadcast across partitions; DMA can't
                    # use a 0-stride SBUF source, but a DRAM source works).
                    dpk = norm_pool.tile([128, 16], F32, tag="dpk")
                    nc.sync.dma_start(
                        dpk[:, 0:8],
                        ctxu_a[64:65, :].rearrange("o (p i) -> o p i", p=128),
                    )
                    nc.sync.dma_start(
                        dpk[:, 8:16],
                        ctxu_b[64:65, :].rearrange("o (p i) -> o p i", p=128),
                    )
                    rpk = norm_pool.tile([128, 16], F32, tag="rpk")
                    nc.vector.reciprocal(rpk[:], dpk[:])
                    nc.sync.dma_start(rscr_pi, rpk[:])
                    bca = norm_pool.tile([64, 1024], F32, tag="bca")
                    nc.gpsimd.dma_start(
                        bca[:].rearrange("d (p i) -> d p i", p=128),
                        rscr_pi[None, :, 0:8].to_broadcast((64, 128, 8)),
                    )
                    bcb = norm_pool.tile([64, 1024], F32, tag="bcb")
                    nc.gpsimd.dma_start(
                        bcb[:].rearrange("d (p i) -> d p i", p=128),
                        rscr_pi[None, :, 8:16].to_broadcast((64, 128, 8)),
                    )
                    nc.vector.tensor_tensor(
                        ctxt_sb[0:64, hp, :], ctxu_a[0:64, :], bca[:],
                        mybir.AluOpType.mult,
                    )
                    tmp_o = norm_pool.tile([64, 1024], BF, tag="tmp")
                    nc.gpsimd.tensor_tensor(
                        tmp_o[:], ctxu_b[0:64, :], bcb[:],
                        mybir.AluOpType.mult,
                    )
                    # lift odd head to partitions 64:128 (DMA can cross
                    # partitions; DVE cannot)
                    nc.gpsimd.dma_start(ctxt_sb[64:128, hp, :], tmp_o[:])

                for hp in range(NH // 2):
                    ha, hb = 2 * hp, 2 * hp + 1
                    # per head: rows 0:64 = ctx^T, row 64 = softmax denominator
                    ctx_a = psB.tile([128, 1024], F32, tag="psB")
                    ctx_b = psB.tile([128, 1024], F32, tag="psB")

                    def emit_pv(kc, et_a, et_b, ctx_a=ctx_a, ctx_b=ctx_b,
                                ha=ha, hb=hb):
                        first, last = kc == 0, kc == LC - 1
                        for qh in range(2):
                            o = qh * 512
                            # ctx^T[d, q] += V^T P^T ; row 64 = denominator
                            nc.tensor.matmul(
                                ctx_a[0 : HD + 1, o : o + 512],
                                v_sb[:, kc, ha, :],
                                et_a[:, o : o + 512],
                                start=first,
                                stop=last,
                            )
                            nc.tensor.matmul(
                                ctx_b[0 : HD + 1, o : o + 512],
                                v_sb[:, kc, hb, :],
                                et_b[:, o : o + 512],
                                start=first,
                                stop=last,
                            )

                    pv_q = []  # software pipeline: PV(kc-1) after ST(kc)
                    for kc in range(LC):
                        # per-head ST then its exp immediately, so ScalarE
                        # starts head a's exp while the PE streams head b
                        st_a = psA.tile([128, 1024], F32, tag="psA")
                        st_b = psA.tile([128, 1024], F32, tag="psA")
                        for o in (0, 512):
                            nc.tensor.matmul(
                                st_a[:, o : o + 512],
                                kt_sb[:, ha, kc * 128 : kc * 128 + 128],
                                qt_sb[:, hp, o : o + 512],
                                start=True,
                                stop=True,
                            )
                        et_a = et_pool.tile([128, 1024], BF, tag="et")
                        nc.scalar.activation(
                            et_a[:], st_a[:], EXP,
                            bias=mask_sb[:, kc : kc + 1], scale=0.125,
                        )
                        for o in (0, 512):
                            nc.tensor.matmul(
                                st_b[:, o : o + 512],
                                kt_sb[:, hb, kc * 128 : kc * 128 + 128],
                                qt_sb[:, hp, o : o + 512],
                                start=True,
                                stop=True,
                            )
                        et_b = et_pool.tile([128, 1024], BF, tag="et")
                        nc.scalar.activation(
                            et_b[:], st_b[:], EXP,
                            bias=mask_sb[:, kc : kc + 1], scale=0.125,
                        )
                        pv_q.append((kc, et_a, et_b))
                        if kc >= 1:
                            emit_pv(*pv_q.pop(0))
                    emit_pv(*pv_q.pop(0))
                    # evacuate ctx+denominator to SBUF immediately: frees the
                    # PSUM slot; normalize runs on DVE/Pool/DMA only and
                    # overlaps the next pair's matmul stream
                    ctxu_a = norm_pool.tile([65, 1024], F32, tag="cua")
                    nc.vector.tensor_copy(ctxu_a[:], ctx_a[0:65, :])
                    ctxu_b = norm_pool.tile([65, 1024], F32, tag="cub")
                    nc.vector.tensor_copy(ctxu_b[:], ctx_b[0:65, :])
                    emit_normalize(hp, ctxu_a, ctxu_b)

                # ---- output projection: out[q, o] = ctx Wo^T + bo ----
                # Emitted split: chunks 0..4 of up to four lc groups stream
                # first (psA+psB pools, 4 open accumulation groups), the
                # chunk-5 matmuls (gated on the last head pair's normalize
                # chain) trail behind — so the PE keeps streaming while the
                # final normalize's DMA ladder completes.
                ps_of = {}

                def op_partial(lc):
                    pool = psA if (lc % 2 == 0) else psB
                    ps = pool.tile([128, 1024], F32, tag=pool is psA and "psA" or "psB")
                    ps_of[lc] = ps
                    for off, width in ((0, 512), (512, 256)):
                        for c in range(NC - 1):
                            nc.tensor.matmul(
                                ps[:, off : off + width],
                                ctxt_sb[:, c, lc * 128 : lc * 128 + 128],
                                wo_sb[:, c, off : off + width],
                                start=(c == 0),
                                stop=False,
                            )

                def op_finish(lc):
                    ps = ps_of.pop(lc)
                    c = NC - 1
                    for off, width in ((0, 512), (512, 256)):
                        nc.tensor.matmul(
                            ps[:, off : off + width],
                            ctxt_sb[:, c, lc * 128 : lc * 128 + 128],
                            wo_sb[:, c, off : off + width],
                            start=False,
                            stop=True,
                        )
                    o_sb = out_pool.tile([128, H], F32, tag="outp")
                    nc.vector.tensor_tensor(
                        o_sb[:], ps[:, 0:H], bo_sb[:], mybir.AluOpType.add
                    )
                    nc.sync.dma_start(out_e[lc * 128 : lc * 128 + 128, :], o_sb[:])

                for lc in range(4):
                    op_partial(lc)
                for lc in range(4, LC):
                    op_finish(lc - 4)
                    op_partial(lc)
                for lc in range(LC - 4, LC):
                    op_finish(lc)

    nc.finalize()
    nc.m = get_hw_module(nc.m)
    return nc


_NC_CACHE = {}


def _get_nc(compute_rounded: bool = True):
    if compute_rounded not in _NC_CACHE:
        _NC_CACHE[compute_rounded] = build_bass(compute_rounded)
    return _NC_CACHE[compute_rounded]


def make_in_maps(inputs):
    f = lambda a: np.ascontiguousarray(np.asarray(a, dtype=np.float32))  # noqa: E731
    fb = lambda a: np.ascontiguousarray(  # noqa: E731
        np.asarray(a, dtype=np.float32).astype(ml_dtypes.bfloat16)
    )
    hs = f(inputs["hidden_states"])
    mask = f(inputs["attention_mask"]).reshape(B, L)
    shared = {
        "wqt": fb(np.asarray(inputs["Wq"]).T),
        "wkt": fb(np.asarray(inputs["Wk"]).T),
        "wvt": fb(np.asarray(inputs["Wv"]).T),
        "wot": fb(np.asarray(inputs["Wo"]).T),
        "bq": f(inputs["bq"]),
        "bk": f(inputs["bk"]),
        "bv": f(inputs["bv"]),
        "bo": f(inputs["bo"]),
    }
    return [
        {"xt": fb(hs[b].T), "mask": mask[b], **shared}
        for b in range(B)
    ]


def run_spmd(inputs, trace=False, compute_rounded=True):
    nc = _get_nc(compute_rounded)
    res = run_bass_kernel_spmd(nc, make_in_maps(inputs), list(range(B)), trace=trace)
    out = np.stack([res.results[b]["out"] for b in range(B)]).astype(np.float32)
    return out, res


def kernel(**inputs) -> np.ndarray:
    out, _ = run_spmd(inputs, trace=False)
    return out


# revision 10
# speedup vs baseline: 1.3192x; 1.2134x over previous
"""BERT self-attention on 8 Trainium2 NeuronCores.

Sharding: data-parallel over batch (B=8 -> one batch element per core).
Each core computes full self-attention for its batch element:
  Q/K/V projections, per-head softmax(Q K^T / 8 + mask) V, output proj.

Layout strategy (per core):
  - Host passes xt = x.T [768,1024] and W.T [768,768] in bf16 so every
    matmul contracts over the partition axis and streams at 1 cyc/row.
  - QT,KT [d, L] and V [L, d] are produced directly by the projections.
  - Attention runs transposed: ST[k,q] = K Q^T per head, so softmax's
    reduction axis (k) lands on partitions: exp via ScalarE with the
    attention mask as per-partition bias (no max subtraction: scores are
    ~N(0,1), |s|<~7, exp is safe in fp32); the denominator comes from a
    ones column appended to V (out row 64); P^T V accumulates ctx^T
    [d, q] which feeds the output projection as lhsT directly.
  - All matmul inputs are bf16 (fp32 PSUM accumulation); measured rel
    err vs the fp32 reference is ~7e-3.
  - The PE does only the structural matmuls: QKV/out biases are folded
    into the PSUM->SBUF evacuation ops against DMA-broadcast bias rows,
    and the softmax 1/denom broadcast is computed entirely off the PE:
    the two denominator rows are DMA-packed across 128 partitions
    ([128,16]), reciprocated in one cheap DVE op, DMA-unpacked, and
    partition-broadcast on GpSimd.  This keeps ScalarE exp-only, which
    is what paces the attention inner loop.
  - Startup: weight chunks load on the Sync HWDGE ring while xt chunks
    load on the Scalar HWDGE ring, halving time-to-first-matmul.
"""

import numpy as np
import ml_dtypes

import concourse.bass as bass  # noqa: F401
import concourse.mybir as mybir
import concourse.tile as tile
from concourse import bacc
from concourse.bass_interp import get_hw_module
from concourse.bass_utils import run_bass_kernel_spmd

B, L, H = 8, 1024, 768
NH, HD = 12, 64
NC = H // 128          # 6 chunks of hidden dim
LC = L // 128          # 8 chunks of sequence dim
F32 = mybir.dt.float32
BF = mybir.dt.bfloat16
EXP = mybir.ActivationFunctionType.Exp


def build_bass(compute_rounded: bool = True):
    del compute_rounded  # single all-bf16 variant

    nc = bacc.Bacc("TRN2", debug=False, num_devices=8)

    xt_e = nc.declare_dram_parameter("xt", [H, L], BF, isOutput=False)
    wqt_e = nc.declare_dram_parameter("wqt", [H, H], BF, isOutput=False)
    wkt_e = nc.declare_dram_parameter("wkt", [H, H], BF, isOutput=False)
    wvt_e = nc.declare_dram_parameter("wvt", [H, H], BF, isOutput=False)
    wot_e = nc.declare_dram_parameter("wot", [H, H], BF, isOutput=False)
    bq_e = nc.declare_dram_parameter("bq", [H], F32, isOutput=False)
    bk_e = nc.declare_dram_parameter("bk", [H], F32, isOutput=False)
    bv_e = nc.declare_dram_parameter("bv", [H], F32, isOutput=False)
    bo_e = nc.declare_dram_parameter("bo", [H], F32, isOutput=False)
    mask_e = nc.declare_dram_parameter("mask", [L], F32, isOutput=False)
    out_e = nc.declare_dram_parameter("out", [L, H], F32, isOutput=True)
    # DRAM scratch for the softmax reciprocal broadcast roundtrip
    rscr_e = nc.dram_tensor("rscr", [2048], F32)

    with tile.TileContext(nc) as tc:
        with (
            tc.tile_pool(name="small", bufs=1) as small,
            tc.tile_pool(name="acts", bufs=1) as acts,
            tc.tile_pool(name="outp", bufs=2) as out_pool,
            tc.tile_pool(name="psA", bufs=2, space="PSUM") as psA,
            tc.tile_pool(name="psB", bufs=2, space="PSUM") as psB,
        ):
            # ---- constants / small tensors (DMAs issued after wv/xt: the
            # V-proj's first matmul gates on wv/xt chunk 0, these don't) ----
            mask_sb = small.tile([128, LC], F32)
            bq_sb = small.tile([128, NC], F32)
            bk_sb = small.tile([128, NC], F32)
            bv_sb = small.tile([128, H], F32)
            bo_sb = small.tile([128, H], F32)

            def load_smalls():
                nc.sync.dma_start(
                    mask_sb[:], mask_e[:].rearrange("(c p) -> p c", p=128)
                )
                nc.sync.dma_start(bq_sb[:], bq_e[:].rearrange("(c p) -> p c", p=128))
                nc.sync.dma_start(bk_sb[:], bk_e[:].rearrange("(c p) -> p c", p=128))
                # bias rows replicated across all partitions (free-dim biases)
                nc.sync.dma_start(bv_sb[:], bv_e[None, :].to_broadcast((128, H)))
                nc.sync.dma_start(bo_sb[:], bo_e[None, :].to_broadcast((128, H)))

            qt_sb = acts.tile([128, NC, L], BF)
            kt_sb = acts.tile([128, NH, L], BF)  # per-head K^T, other 64 rows zero
            nc.gpsimd.memset(kt_sb[:], 0.0)
            v_sb = acts.tile([128, LC, NH, HD + 1], BF)  # [..., 64] = ones col
            ctxt_sb = acts.tile([128, NC, L], BF)
            nc.vector.memset(v_sb[:, :, :, HD], 1.0)

            # =========== projection phase (xt + wv/wk/wq scoped) ===========
            with (
                tc.tile_pool(name="xt", bufs=1) as xt_pool,
                tc.tile_pool(name="w1", bufs=2) as w1,
            ):
                # weights ride the Sync HWDGE ring, xt the Scalar ring, so
                # the V-proj's first matmul only waits ~one chunk per ring
                xt_sb = xt_pool.tile([128, NC, L], BF)
                wv_sb = w1.tile([128, NC, H], BF, tag="w")
                for c in range(NC):
                    eng_w = nc.sync if c % 2 == 0 else nc.scalar
                    eng_x = nc.scalar if c % 2 == 0 else nc.sync
                    eng_w.dma_start(
                        wv_sb[:, c, :],
                        wvt_e[:].rearrange("(c p) d -> p c d", p=128)[:, c, :],
                    )
                    eng_x.dma_start(
                        xt_sb[:, c, :],
                        xt_e[:].rearrange("(c p) q -> p c q", p=128)[:, c, :],
                    )
                load_smalls()
                for lc in range(LC):
                    ps = psA.tile([128, 1024], F32, tag="psA")
                    for off, width in ((0, 512), (512, 256)):
                        for kc in range(NC):
                            nc.tensor.matmul(
                                ps[:, off : off + width],
                                xt_sb[:, kc, lc * 128 : lc * 128 + 128],
                                wv_sb[:, kc, off : off + width],
                                start=(kc == 0),
                                stop=(kc == NC - 1),
                            )
                    # evacuate + bv add (bias varies along free dim)
                    nc.vector.tensor_tensor(
                        v_sb[:, lc, :, 0:HD],
                        ps[:, 0:H].rearrange("p (h d) -> p h d", d=HD),
                        bv_sb[:].rearrange("p (h d) -> p h d", d=HD),
                        mybir.AluOpType.add,
                    )

                # ---- K^T then Q^T projections: out[d, q] = W x^T + b
                for w_e, b_sb, dst in ((wkt_e, bk_sb, kt_sb), (wqt_e, bq_sb, qt_sb)):
                    w_sb = w1.tile([128, NC, H], BF, tag="w")
                    nc.sync.dma_start(
                        w_sb[:], w_e[:].rearrange("(c p) d -> p c d", p=128)
                    )
                    for dc in range(NC):
                        ps = psA.tile([128, 1024], F32, tag="psA")
                        for qh in range(2):
                            o = qh * 512
                            for kc in range(NC):
                                nc.tensor.matmul(
                                    ps[:, o : o + 512],
                                    w_sb[:, kc, dc * 128 : dc * 128 + 128],
                                    xt_sb[:, kc, o : o + 512],
                                    start=(kc == 0),
                                    stop=(kc == NC - 1),
                                )
                        if dst is qt_sb:
                            nc.vector.tensor_scalar_add(
                                dst[:, dc, :], ps[:, :], b_sb[:, dc : dc + 1]
                            )
                        else:
                            nc.vector.tensor_scalar_add(
                                kt_sb[0:64, 2 * dc, :], ps[0:64, :],
                                b_sb[0:64, dc : dc + 1],
                            )
                            nc.vector.tensor_scalar_add(
                                kt_sb[64:128, 2 * dc + 1, :], ps[64:128, :],
                                b_sb[64:128, dc : dc + 1],
                            )

            # =========== attention + output projection ===========
            with (
                tc.tile_pool(name="w2", bufs=1) as w2,
                tc.tile_pool(name="et", bufs=6) as et_pool,
                tc.tile_pool(name="norm", bufs=1) as norm_pool,
            ):
                wo_sb = w2.tile([128, NC, H], BF)
                nc.sync.dma_start(
                    wo_sb[:], wot_e[:].rearrange("(c p) d -> p c d", p=128)
                )

                rscr_pi = rscr_e[:].rearrange("(p i) -> p i", p=128)

                def emit_normalize(hp, ctxu_a, ctxu_b):
                    # softmax 1/denom, entirely off the PE/ACT engines:
                    # pack both denominator rows across 128 partitions
                    # (q = p*8+i), one cheap DVE reciprocal, then a DRAM
                    # roundtrip to replicate the reciprocals to 64 rows
                    # (engines can't broadcast across partitions; DMA can't
                    # use a 0-stride SBUF source, but a DRAM source works).
                    dpk = norm_pool.tile([128, 16], F32, tag="dpk")
                    nc.sync.dma_start(
                        dpk[:, 0:8],
                        ctxu_a[64:65, :].rearrange("o (p i) -> o p i", p=128),
                    )
                    nc.sync.dma_start(
                        dpk[:, 8:16],
                        ctxu_b[64:65, :].rearrange("o (p i) -> o p i", p=128),
                    )
                    rpk = norm_pool.tile([128, 16], F32, tag="rpk")
                    nc.vector.reciprocal(rpk[:], dpk[:])
                    nc.sync.dma_start(rscr_pi, rpk[:])
                    bca = norm_pool.tile([64, 1024], F32, tag="bca")
                    nc.sync.dma_start(
                        bca[:].rearrange("d (p i) -> d p i", p=128),
                        rscr_pi[None, :, 0:8].to_broadcast((64, 128, 8)),
                    )
                    bcb = norm_pool.tile([64, 1024], F32, tag="bcb")
                    nc.sync.dma_start(
                        bcb[:].rearrange("d (p i) -> d p i", p=128),
                        rscr_pi[None, :, 8:16].to_broadcast((64, 128, 8)),
                    )
                    nc.vector.tensor_tensor(
                        ctxt_sb[0:64, hp, :], ctxu_a[0:64, :], bca[:],
                        mybir.AluOpType.mult,
                    )
                    tmp_o = norm_pool.tile([64, 1024], BF, tag="tmp")
                    nc.gpsimd.tensor_tensor(
                        tmp_o[:], ctxu_b[0:64, :], bcb[:],
                        mybir.AluOpType.mult,
                    )
                    # lift odd head to partitions 64:128 (DMA can cross
                    # partitions; DVE cannot)
                    nc.sync.dma_start(ctxt_sb[64:128, hp, :], tmp_o[:])

                for hp in range(NH // 2):
                    ha, hb = 2 * hp, 2 * hp + 1
                    # per head: rows 0:64 = ctx^T, row 64 = softmax denominator
                    ctx_a = psB.tile([128, 1024], F32, tag="psB")
                    ctx_b = psB.tile([128, 1024], F32, tag="psB")

                    def emit_pv(kc, et_a, et_b, ctx_a=ctx_a, ctx_b=ctx_b,
                                ha=ha, hb=hb):
                        first, last = kc == 0, kc == LC - 1
                        for qh in range(2):
                            o = qh * 512
                            # ctx^T[d, q] += V^T P^T ; row 64 = denominator
                            nc.tensor.matmul(
                                ctx_a[0 : HD + 1, o : o + 512],
                                v_sb[:, kc, ha, :],
                                et_a[:, o : o + 512],
                                start=first,
                                stop=last,
                            )
                            nc.tensor.matmul(
                                ctx_b[0 : HD + 1, o : o + 512],
                                v_sb[:, kc, hb, :],
                                et_b[:, o : o + 512],
                                start=first,
                                stop=last,
                            )

                    pv_q = []  # software pipeline: PV(kc-1) after ST(kc)
                    for kc in range(LC):
                        # per-head ST then its exp immediately, so ScalarE
                        # starts head a's exp while the PE streams head b
                        st_a = psA.tile([128, 1024], F32, tag="psA")
                        st_b = psA.tile([128, 1024], F32, tag="psA")
                        for o in (0, 512):
                            nc.tensor.matmul(
                                st_a[:, o : o + 512],
                                kt_sb[:, ha, kc * 128 : kc * 128 + 128],
                                qt_sb[:, hp, o : o + 512],
                                start=True,
                                stop=True,
                            )
                        et_a = et_pool.tile([128, 1024], BF, tag="et")
                        nc.scalar.activation(
                            et_a[:], st_a[:], EXP,
                            bias=mask_sb[:, kc : kc + 1], scale=0.125,
                        )
                        for o in (0, 512):
                            nc.tensor.matmul(
                                st_b[:, o : o + 512],
                                kt_sb[:, hb, kc * 128 : kc * 128 + 128],
                                qt_sb[:, hp, o : o + 512],
                                start=True,
                                stop=True,
                            )
                        et_b = et_pool.tile([128, 1024], BF, tag="et")
                        nc.scalar.activation(
                            et_b[:], st_b[:], EXP,
                            bias=mask_sb[:, kc : kc + 1], scale=0.125,
                        )
                        pv_q.append((kc, et_a, et_b))
                        if kc >= 1:
                            emit_pv(*pv_q.pop(0))
                    emit_pv(*pv_q.pop(0))
                    # evacuate ctx+denominator to SBUF immediately: frees the
                    # PSUM slot; normalize runs on DVE/Pool/DMA only and
                    # overlaps the next pair's matmul stream
                    ctxu_a = norm_pool.tile([65, 1024], F32, tag="cua")
                    nc.vector.tensor_copy(ctxu_a[:], ctx_a[0:65, :])
                    ctxu_b = norm_pool.tile([65, 1024], F32, tag="cub")
                    nc.vector.tensor_copy(ctxu_b[:], ctx_b[0:65, :])
                    emit_normalize(hp, ctxu_a, ctxu_b)

                # ---- output projection: out[q, o] = ctx Wo^T + bo ----
                # Emitted split: chunks 0..4 of up to four lc groups stream
                # first (psA+psB pools, 4 open accumulation groups), the
                # chunk-5 matmuls (gated on the last head pair's normalize
                # chain) trail behind — so the PE keeps streaming while the
                # final normalize's DMA ladder completes.
                ps_of = {}

                def op_partial(lc):
                    pool = psA if (lc % 2 == 0) else psB
                    ps = pool.tile([128, 1024], F32, tag=pool is psA and "psA" or "psB")
                    ps_of[lc] = ps
                    for off, width in ((0, 512), (512, 256)):
                        for c in range(NC - 1):
                            nc.tensor.matmul(
                                ps[:, off : off + width],
                                ctxt_sb[:, c, lc * 128 : lc * 128 + 128],
                                wo_sb[:, c, off : off + width],
                                start=(c == 0),
                                stop=False,
                            )

                def op_finish(lc):
                    ps = ps_of.pop(lc)
                    c = NC - 1
                    for off, width in ((0, 512), (512, 256)):
                        nc.tensor.matmul(
                            ps[:, off : off + width],
                            ctxt_sb[:, c, lc * 128 : lc * 128 + 128],
                            wo_sb[:, c, off : off + width],
                            start=False,
                            stop=True,
                        )
                    o_sb = out_pool.tile([128, H], F32, tag="outp")
                    nc.vector.tensor_tensor(
                        o_sb[:], ps[:, 0:H], bo_sb[:], mybir.AluOpType.add
                    )
                    nc.sync.dma_start(out_e[lc * 128 : lc * 128 + 128, :], o_sb[:])

                for lc in range(4):
                    op_partial(lc)
                for lc in range(4, LC):
                    op_finish(lc - 4)
                    op_partial(lc)
                for lc in range(LC - 4, LC):
                    op_finish(lc)

    nc.finalize()
    nc.m = get_hw_module(nc.m)
    return nc


_NC_CACHE = {}


def _get_nc(compute_rounded: bool = True):
    if compute_rounded not in _NC_CACHE:
        _NC_CACHE[compute_rounded] = build_bass(compute_rounded)
    return _NC_CACHE[compute_rounded]


def make_in_maps(inputs):
    f = lambda a: np.ascontiguousarray(np.asarray(a, dtype=np.float32))  # noqa: E731
    fb = lambda a: np.ascontiguousarray(  # noqa: E731
        np.asarray(a, dtype=np.float32).astype(ml_dtypes.bfloat16)
    )
    hs = f(inputs["hidden_states"])
    mask = f(inputs["attention_mask"]).reshape(B, L)
    shared = {
        "wqt": fb(np.asarray(inputs["Wq"]).T),
        "wkt": fb(np.asarray(inputs["Wk"]).T),
        "wvt": fb(np.asarray(inputs["Wv"]).T),
        "wot": fb(np.asarray(inputs["Wo"]).T),
        "bq": f(inputs["bq"]),
        "bk": f(inputs["bk"]),
        "bv": f(inputs["bv"]),
        "bo": f(inputs["bo"]),
    }
    return [
        {"xt": fb(hs[b].T), "mask": mask[b], **shared}
        for b in range(B)
    ]


def run_spmd(inputs, trace=False, compute_rounded=True):
    nc = _get_nc(compute_rounded)
    res = run_bass_kernel_spmd(nc, make_in_maps(inputs), list(range(B)), trace=trace)
    out = np.stack([res.results[b]["out"] for b in range(B)]).astype(np.float32)
    return out, res


def kernel(**inputs) -> np.ndarray:
    out, _ = run_spmd(inputs, trace=False)
    return out


# revision 13
# speedup vs baseline: 1.3332x; 1.0106x over previous
"""BERT self-attention on 8 Trainium2 NeuronCores.

Sharding: data-parallel over batch (B=8 -> one batch element per core).
Each core computes full self-attention for its batch element:
  Q/K/V projections, per-head softmax(Q K^T / 8 + mask) V, output proj.

Layout strategy (per core):
  - Host passes xt = x.T [768,1024] and W.T [768,768] in bf16 so every
    matmul contracts over the partition axis and streams at 1 cyc/row.
  - QT,KT [d, L] and V [L, d] are produced directly by the projections.
  - Attention runs transposed: ST[k,q] = K Q^T per head, so softmax's
    reduction axis (k) lands on partitions: exp via ScalarE with the
    attention mask as per-partition bias (no max subtraction: scores are
    ~N(0,1), |s|<~7, exp is safe in fp32); the denominator comes from a
    ones column appended to V (out row 64); P^T V accumulates ctx^T
    [d, q] which feeds the output projection as lhsT directly.
  - All matmul inputs are bf16 (fp32 PSUM accumulation); measured rel
    err vs the fp32 reference is ~7e-3.
  - The PE does only the structural matmuls: QKV/out biases are folded
    into the PSUM->SBUF evacuation ops against DMA-broadcast bias rows,
    and the softmax 1/denom broadcast is computed entirely off the PE:
    the two denominator rows are DMA-packed across 128 partitions
    ([128,16]), reciprocated in one cheap DVE op, DMA-unpacked, and
    partition-broadcast on GpSimd.  This keeps ScalarE exp-only, which
    is what paces the attention inner loop.
  - Startup: weight chunks load on the Sync HWDGE ring while xt chunks
    load on the Scalar HWDGE ring, halving time-to-first-matmul.
"""

import numpy as np
import ml_dtypes

import concourse.bass as bass  # noqa: F401
import concourse.mybir as mybir
import concourse.tile as tile
from concourse import bacc
from concourse.bass_interp import get_hw_module
from concourse.bass_utils import run_bass_kernel_spmd

B, L, H = 8, 1024, 768
NH, HD = 12, 64
NC = H // 128          # 6 chunks of hidden dim
LC = L // 128          # 8 chunks of sequence dim
F32 = mybir.dt.float32
BF = mybir.dt.bfloat16
EXP = mybir.ActivationFunctionType.Exp


def build_bass(compute_rounded: bool = True):
    del compute_rounded  # single all-bf16 variant

    nc = bacc.Bacc("TRN2", debug=False, num_devices=8)

    xt_e = nc.declare_dram_parameter("xt", [H, L], BF, isOutput=False)
    wqt_e = nc.declare_dram_parameter("wqt", [H, H], BF, isOutput=False)
    wkt_e = nc.declare_dram_parameter("wkt", [H, H], BF, isOutput=False)
    wvt_e = nc.declare_dram_parameter("wvt", [H, H], BF, isOutput=False)
    wot_e = nc.declare_dram_parameter("wot", [H, H], BF, isOutput=False)
    bq_e = nc.declare_dram_parameter("bq", [H], F32, isOutput=False)
    bk_e = nc.declare_dram_parameter("bk", [H], F32, isOutput=False)
    bv_e = nc.declare_dram_parameter("bv", [H], F32, isOutput=False)
    bo_e = nc.declare_dram_parameter("bo", [H], F32, isOutput=False)
    mask_e = nc.declare_dram_parameter("mask", [L], F32, isOutput=False)
    out_e = nc.declare_dram_parameter("out", [L, H], F32, isOutput=True)
    # DRAM scratch for the softmax reciprocal broadcast roundtrip
    rscr_e = nc.dram_tensor("rscr", [2048], F32)

    with tile.TileContext(nc) as tc:
        with (
            tc.tile_pool(name="small", bufs=1) as small,
            tc.tile_pool(name="acts", bufs=1) as acts,
            tc.tile_pool(name="outp", bufs=2) as out_pool,
            tc.tile_pool(name="psA", bufs=2, space="PSUM") as psA,
            tc.tile_pool(name="psB", bufs=2, space="PSUM") as psB,
        ):
            # ---- constants / small tensors (DMAs issued after wv/xt: the
            # V-proj's first matmul gates on wv/xt chunk 0, these don't) ----
            mask_sb = small.tile([128, LC], F32)
            bq_sb = small.tile([128, NC], F32)
            bk_sb = small.tile([128, NC], F32)
            bv_sb = small.tile([128, H], F32)
            bo_sb = small.tile([128, H], F32)

            def load_smalls():
                nc.sync.dma_start(
                    mask_sb[:], mask_e[:].rearrange("(c p) -> p c", p=128)
                )
                nc.sync.dma_start(bq_sb[:], bq_e[:].rearrange("(c p) -> p c", p=128))
                nc.sync.dma_start(bk_sb[:], bk_e[:].rearrange("(c p) -> p c", p=128))
                # bias rows replicated across all partitions (free-dim biases)
                nc.sync.dma_start(bv_sb[:], bv_e[None, :].to_broadcast((128, H)))
                nc.sync.dma_start(bo_sb[:], bo_e[None, :].to_broadcast((128, H)))

            qt_sb = acts.tile([128, NC, L], BF)
            kt_sb = acts.tile([128, NH, L], BF)  # per-head K^T, other 64 rows zero
            nc.gpsimd.memset(kt_sb[:], 0.0)
            v_sb = acts.tile([128, LC, NH, HD + 1], BF)  # [..., 64] = ones col
            ctxt_sb = acts.tile([128, NC, L], BF)
            nc.vector.memset(v_sb[:, :, :, HD], 1.0)

            # =========== projection phase (xt + wv/wk/wq scoped) ===========
            with (
                tc.tile_pool(name="xt", bufs=1) as xt_pool,
                tc.tile_pool(name="w1", bufs=2) as w1,
            ):
                # weights ride the Sync HWDGE ring, xt the Scalar ring, so
                # the V-proj's first matmul only waits ~one chunk per ring
                xt_sb = xt_pool.tile([128, NC, L], BF)
                wv_sb = w1.tile([128, NC, H], BF, tag="w")
                for c in range(NC):
                    eng_w = nc.sync if c % 2 == 0 else nc.scalar
                    eng_x = nc.scalar if c % 2 == 0 else nc.sync
                    eng_w.dma_start(
                        wv_sb[:, c, :],
                        wvt_e[:].rearrange("(c p) d -> p c d", p=128)[:, c, :],
                    )
                    eng_x.dma_start(
                        xt_sb[:, c, :],
                        xt_e[:].rearrange("(c p) q -> p c q", p=128)[:, c, :],
                    )
                load_smalls()
                for lc in range(LC):
                    ps = psA.tile([128, 1024], F32, tag="psA")
                    for off, width in ((0, 512), (512, 256)):
                        for kc in range(NC):
                            nc.tensor.matmul(
                                ps[:, off : off + width],
                                xt_sb[:, kc, lc * 128 : lc * 128 + 128],
                                wv_sb[:, kc, off : off + width],
                                start=(kc == 0),
                                stop=(kc == NC - 1),
                            )
                    # evacuate + bv add (bias varies along free dim)
                    nc.vector.tensor_tensor(
                        v_sb[:, lc, :, 0:HD],
                        ps[:, 0:H].rearrange("p (h d) -> p h d", d=HD),
                        bv_sb[:].rearrange("p (h d) -> p h d", d=HD),
                        mybir.AluOpType.add,
                    )

                # ---- K^T then Q^T projections: out[d, q] = W x^T + b
                for w_e, b_sb, dst in ((wkt_e, bk_sb, kt_sb), (wqt_e, bq_sb, qt_sb)):
                    w_sb = w1.tile([128, NC, H], BF, tag="w")
                    nc.sync.dma_start(
                        w_sb[:], w_e[:].rearrange("(c p) d -> p c d", p=128)
                    )
                    for dc in range(NC):
                        ps = psA.tile([128, 1024], F32, tag="psA")
                        for qh in range(2):
                            o = qh * 512
                            for kc in range(NC):
                                nc.tensor.matmul(
                                    ps[:, o : o + 512],
                                    w_sb[:, kc, dc * 128 : dc * 128 + 128],
                                    xt_sb[:, kc, o : o + 512],
                                    start=(kc == 0),
                                    stop=(kc == NC - 1),
                                )
                        if dst is qt_sb:
                            nc.vector.tensor_scalar_add(
                                dst[:, dc, :], ps[:, :], b_sb[:, dc : dc + 1]
                            )
                        else:
                            nc.vector.tensor_scalar_add(
                                kt_sb[0:64, 2 * dc, :], ps[0:64, :],
                                b_sb[0:64, dc : dc + 1],
                            )
                            nc.vector.tensor_scalar_add(
                                kt_sb[64:128, 2 * dc + 1, :], ps[64:128, :],
                                b_sb[64:128, dc : dc + 1],
                            )

            # =========== attention + output projection ===========
            with (
                tc.tile_pool(name="w2", bufs=1) as w2,
                tc.tile_pool(name="et", bufs=6) as et_pool,
                tc.tile_pool(name="norm", bufs=1) as norm_pool,
            ):
                wo_sb = w2.tile([128, NC, H], BF)
                nc.sync.dma_start(
                    wo_sb[:], wot_e[:].rearrange("(c p) d -> p c d", p=128)
                )

                def emit_normalize(hp, ctxu):
                    # softmax 1/denom, entirely off the PE/ACT engines:
                    # pack both heads' denominator rows across 128 partitions
                    # (q = p*8+i per head), one cheap DVE reciprocal, then a
                    # DRAM roundtrip to replicate the reciprocals to 64 rows
                    # (engines can't broadcast across partitions; DMA can't
                    # use a 0-stride SBUF source, but a DRAM source works).
                    dpk = norm_pool.tile([128, 16], F32, tag="dpk")
                    nc.sync.dma_start(
                        dpk[:, 0:8],
                        ctxu[64:65, 0:1024].rearrange("o (p i) -> o p i", p=128),
                    )
                    nc.sync.dma_start(
                        dpk[:, 8:16],
                        ctxu[64:65, 1024:2048].rearrange("o (p i) -> o p i", p=128),
                    )
                    rpk = norm_pool.tile([128, 16], F32, tag="rpk")
                    nc.vector.reciprocal(rpk[:], dpk[:])
                    nc.sync.dma_start(
                        rscr_e[:].rearrange("(p x) -> p x", p=128), rpk[:]
                    )
                    bc = norm_pool.tile([64, 2048], F32, tag="bc")
                    rscr_pi = rscr_e[:].rearrange("(p x) -> p x", p=128)
                    nc.sync.dma_start(
                        bc[:, 0:1024].rearrange("d (p i) -> d p i", p=128),
                        rscr_pi[None, :, 0:8].to_broadcast((64, 128, 8)),
                    )
                    nc.sync.dma_start(
                        bc[:, 1024:2048].rearrange("d (p i) -> d p i", p=128),
                        rscr_pi[None, :, 8:16].to_broadcast((64, 128, 8)),
                    )
                    nc.vector.tensor_tensor(
                        ctxt_sb[0:64, hp, :], ctxu[0:64, 0:1024], bc[:, 0:1024],
                        mybir.AluOpType.mult,
                    )
                    tmp_o = norm_pool.tile([64, 1024], BF, tag="tmp")
                    nc.gpsimd.tensor_tensor(
                        tmp_o[:], ctxu[0:64, 1024:2048], bc[:, 1024:2048],
                        mybir.AluOpType.mult,
                    )
                    # lift odd head to partitions 64:128 (DMA can cross
                    # partitions; DVE cannot)
                    nc.sync.dma_start(ctxt_sb[64:128, hp, :], tmp_o[:])

                for hp in range(NH // 2):
                    ha, hb = 2 * hp, 2 * hp + 1
                    # per head: rows 0:64 = ctx^T, row 64 = softmax denominator
                    ctx_a = psB.tile([128, 1024], F32, tag="psB")
                    ctx_b = psB.tile([128, 1024], F32, tag="psB")

                    def emit_pv(kc, et_a, et_b, ctx_a=ctx_a, ctx_b=ctx_b,
                                ha=ha, hb=hb):
                        first, last = kc == 0, kc == LC - 1
                        for qh in range(2):
                            o = qh * 512
                            # ctx^T[d, q] += V^T P^T ; row 64 = denominator
                            nc.tensor.matmul(
                                ctx_a[0 : HD + 1, o : o + 512],
                                v_sb[:, kc, ha, :],
                                et_a[:, o : o + 512],
                                start=first,
                                stop=last,
                            )
                            nc.tensor.matmul(
                                ctx_b[0 : HD + 1, o : o + 512],
                                v_sb[:, kc, hb, :],
                                et_b[:, o : o + 512],
                                start=first,
                                stop=last,
                            )

                    pv_q = []  # software pipeline: PV(kc-1) after ST(kc)
                    for kc in range(LC):
                        # per-head ST then its exp immediately, so ScalarE
                        # starts head a's exp while the PE streams head b
                        st_a = psA.tile([128, 1024], F32, tag="psA")
                        st_b = psA.tile([128, 1024], F32, tag="psA")
                        for o in (0, 512):
                            nc.tensor.matmul(
                                st_a[:, o : o + 512],
                                kt_sb[:, ha, kc * 128 : kc * 128 + 128],
                                qt_sb[:, hp, o : o + 512],
                                start=True,
                                stop=True,
                            )
                        et_a = et_pool.tile([128, 1024], BF, tag="et")
                        nc.scalar.activation(
                            et_a[:], st_a[:], EXP,
                            bias=mask_sb[:, kc : kc + 1], scale=0.125,
                        )
                        for o in (0, 512):
                            nc.tensor.matmul(
                                st_b[:, o : o + 512],
                                kt_sb[:, hb, kc * 128 : kc * 128 + 128],
                                qt_sb[:, hp, o : o + 512],
                                start=True,
                                stop=True,
                            )
                        et_b = et_pool.tile([128, 1024], BF, tag="et")
                        nc.scalar.activation(
                            et_b[:], st_b[:], EXP,
                            bias=mask_sb[:, kc : kc + 1], scale=0.125,
                        )
                        pv_q.append((kc, et_a, et_b))
                        if kc >= 1:
                            emit_pv(*pv_q.pop(0))
                    emit_pv(*pv_q.pop(0))
                    # evacuate ctx+denominator to SBUF immediately: frees the
                    # PSUM slots for the next pair.  Head a evacuates on DVE,
                    # head b on ScalarE (activation Copy; GpSimd can't read
                    # PSUM per the BIR verifier), so the two copies run in
                    # parallel and neither queues behind the previous pair's
                    # normalize chain.
                    ctxu = norm_pool.tile([65, 2048], F32, tag="cu")
                    nc.vector.tensor_copy(ctxu[:, 0:1024], ctx_a[0:65, :])
                    nc.scalar.activation(
                        ctxu[:, 1024:2048], ctx_b[0:65, :],
                        mybir.ActivationFunctionType.Copy,
                    )
                    emit_normalize(hp, ctxu)

                # ---- output projection: out[q, o] = ctx Wo^T + bo ----
                # Emitted split: chunks 0..4 of up to four lc groups stream
                # first (psA+psB pools, 4 open accumulation groups), the
                # chunk-5 matmuls (gated on the last head pair's normalize
                # chain) trail behind — so the PE keeps streaming while the
                # final normalize's DMA ladder completes.
                ps_of = {}

                def op_partial(lc):
                    pool = psA if (lc % 2 == 0) else psB
                    ps = pool.tile([128, 1024], F32, tag=pool is psA and "psA" or "psB")
                    ps_of[lc] = ps
                    for off, width in ((0, 512), (512, 256)):
                        for c in range(NC - 1):
                            nc.tensor.matmul(
                                ps[:, off : off + width],
                                ctxt_sb[:, c, lc * 128 : lc * 128 + 128],
                                wo_sb[:, c, off : off + width],
                                start=(c == 0),
                                stop=False,
                            )

                def op_finish(lc):
                    ps = ps_of.pop(lc)
                    c = NC - 1
                    for off, width in ((0, 512), (512, 256)):
                        nc.tensor.matmul(
                            ps[:, off : off + width],
                            ctxt_sb[:, c, lc * 128 : lc * 128 + 128],
                            wo_sb[:, c, off : off + width],
                            start=False,
                            stop=True,
                        )
                    o_sb = out_pool.tile([128, H], F32, tag="outp")
                    nc.vector.tensor_tensor(
                        o_sb[:], ps[:, 0:H], bo_sb[:], mybir.AluOpType.add
                    )
                    nc.sync.dma_start(out_e[lc * 128 : lc * 128 + 128, :], o_sb[:])

                for lc in range(4):
                    op_partial(lc)
                for lc in range(4, LC):
                    op_finish(lc - 4)
                    op_partial(lc)
                for lc in range(LC - 4, LC):
                    op_finish(lc)

    nc.finalize()
    nc.m = get_hw_module(nc.m)
    return nc


_NC_CACHE = {}


def _get_nc(compute_rounded: bool = True):
    if compute_rounded not in _NC_CACHE:
        _NC_CACHE[compute_rounded] = build_bass(compute_rounded)
    return _NC_CACHE[compute_rounded]


def make_in_maps(inputs):
    f = lambda a: np.ascontiguousarray(np.asarray(a, dtype=np.float32))  # noqa: E731
    fb = lambda a: np.ascontiguousarray(  # noqa: E731
        np.asarray(a, dtype=np.float32).astype(ml_dtypes.bfloat16)
    )
    hs = f(inputs["hidden_states"])
    mask = f(inputs["attention_mask"]).reshape(B, L)
    shared = {
        "wqt": fb(np.asarray(inputs["Wq"]).T),
        "wkt": fb(np.asarray(inputs["Wk"]).T),
        "wvt": fb(np.asarray(inputs["Wv"]).T),
        "wot": fb(np.asarray(inputs["Wo"]).T),
        "bq": f(inputs["bq"]),
        "bk": f(inputs["bk"]),
        "bv": f(inputs["bv"]),
        "bo": f(inputs["bo"]),
    }
    return [
        {"xt": fb(hs[b].T), "mask": mask[b], **shared}
        for b in range(B)
    ]


def run_spmd(inputs, trace=False, compute_rounded=True):
    nc = _get_nc(compute_rounded)
    res = run_bass_kernel_spmd(nc, make_in_maps(inputs), list(range(B)), trace=trace)
    out = np.stack([res.results[b]["out"] for b in range(B)]).astype(np.float32)
    return out, res


def kernel(**inputs) -> np.ndarray:
    out, _ = run_spmd(inputs, trace=False)
    return out


# revision 14
# speedup vs baseline: 1.3346x; 1.0011x over previous
"""BERT self-attention on 8 Trainium2 NeuronCores.

Sharding: data-parallel over batch (B=8 -> one batch element per core).
Each core computes full self-attention for its batch element:
  Q/K/V projections, per-head softmax(Q K^T / 8 + mask) V, output proj.

Layout strategy (per core):
  - Host passes xt = x.T [768,1024] and W.T [768,768] in bf16 so every
    matmul contracts over the partition axis and streams at 1 cyc/row.
  - QT,KT [d, L] and V [L, d] are produced directly by the projections.
  - Attention runs transposed: ST[k,q] = K Q^T per head, so softmax's
    reduction axis (k) lands on partitions: exp via ScalarE with the
    attention mask as per-partition bias (no max subtraction: scores are
    ~N(0,1), |s|<~7, exp is safe in fp32); the denominator comes from a
    ones column appended to V (out row 64); P^T V accumulates ctx^T
    [d, q] which feeds the output projection as lhsT directly.
  - All matmul inputs are bf16 (fp32 PSUM accumulation); measured rel
    err vs the fp32 reference is ~7e-3.
  - The PE does only the structural matmuls: QKV/out biases are folded
    into the PSUM->SBUF evacuation ops against DMA-broadcast bias rows,
    and the softmax 1/denom broadcast is computed entirely off the PE:
    the two denominator rows are DMA-packed across 128 partitions
    ([128,16]), reciprocated in one cheap DVE op, DMA-unpacked, and
    partition-broadcast on GpSimd.  This keeps ScalarE exp-only, which
    is what paces the attention inner loop.
  - Startup: weight chunks load on the Sync HWDGE ring while xt chunks
    load on the Scalar HWDGE ring, halving time-to-first-matmul.
"""

import numpy as np
import ml_dtypes

import concourse.bass as bass  # noqa: F401
import concourse.mybir as mybir
import concourse.tile as tile
from concourse import bacc
from concourse.bass_interp import get_hw_module
from concourse.bass_utils import run_bass_kernel_spmd

B, L, H = 8, 1024, 768
NH, HD = 12, 64
NC = H // 128          # 6 chunks of hidden dim
LC = L // 128          # 8 chunks of sequence dim
F32 = mybir.dt.float32
BF = mybir.dt.bfloat16
EXP = mybir.ActivationFunctionType.Exp


def build_bass(compute_rounded: bool = True):
    del compute_rounded  # single all-bf16 variant

    nc = bacc.Bacc("TRN2", debug=False, num_devices=8)

    xt_e = nc.declare_dram_parameter("xt", [H, L], BF, isOutput=False)
    wqt_e = nc.declare_dram_parameter("wqt", [H, H], BF, isOutput=False)
    wkt_e = nc.declare_dram_parameter("wkt", [H, H], BF, isOutput=False)
    wvt_e = nc.declare_dram_parameter("wvt", [H, H], BF, isOutput=False)
    wot_e = nc.declare_dram_parameter("wot", [H, H], BF, isOutput=False)
    bq_e = nc.declare_dram_parameter("bq", [H], F32, isOutput=False)
    bk_e = nc.declare_dram_parameter("bk", [H], F32, isOutput=False)
    bv_e = nc.declare_dram_parameter("bv", [H], F32, isOutput=False)
    bo_e = nc.declare_dram_parameter("bo", [H], F32, isOutput=False)
    mask_e = nc.declare_dram_parameter("mask", [L], F32, isOutput=False)
    out_e = nc.declare_dram_parameter("out", [L, H], F32, isOutput=True)
    # DRAM scratch for the softmax reciprocal broadcast roundtrip
    rscr_e = nc.dram_tensor("rscr", [2048], F32)

    with tile.TileContext(nc) as tc:
        with (
            tc.tile_pool(name="small", bufs=1) as small,
            tc.tile_pool(name="acts", bufs=1) as acts,
            tc.tile_pool(name="outp", bufs=2) as out_pool,
            tc.tile_pool(name="psA", bufs=2, space="PSUM") as psA,
            tc.tile_pool(name="psB", bufs=2, space="PSUM") as psB,
        ):
            # ---- constants / small tensors (DMAs issued after wv/xt: the
            # V-proj's first matmul gates on wv/xt chunk 0, these don't) ----
            mask_sb = small.tile([128, LC], F32)
            bq_sb = small.tile([128, NC], F32)
            bk_sb = small.tile([128, NC], F32)
            bv_sb = small.tile([128, H], F32)
            bo_sb = small.tile([128, H], F32)

            def load_smalls():
                nc.sync.dma_start(
                    mask_sb[:], mask_e[:].rearrange("(c p) -> p c", p=128)
                )
                nc.sync.dma_start(bq_sb[:], bq_e[:].rearrange("(c p) -> p c", p=128))
                nc.sync.dma_start(bk_sb[:], bk_e[:].rearrange("(c p) -> p c", p=128))
                # bias rows replicated across all partitions (free-dim biases)
                nc.sync.dma_start(bv_sb[:], bv_e[None, :].to_broadcast((128, H)))
                nc.sync.dma_start(bo_sb[:], bo_e[None, :].to_broadcast((128, H)))

            qt_sb = acts.tile([128, NC, L], BF)
            kt_sb = acts.tile([128, NH, L], BF)  # per-head K^T, other 64 rows zero
            nc.gpsimd.memset(kt_sb[:], 0.0)
            v_sb = acts.tile([128, LC, NH, HD + 1], BF)  # [..., 64] = ones col
            ctxt_sb = acts.tile([128, NC, L], BF)
            nc.vector.memset(v_sb[:, :, :, HD], 1.0)

            # =========== projection phase (xt + wv/wk/wq scoped) ===========
            with (
                tc.tile_pool(name="xt", bufs=1) as xt_pool,
                tc.tile_pool(name="w1", bufs=2) as w1,
            ):
                # weights ride the Sync HWDGE ring, xt the Scalar ring, so
                # the V-proj's first matmul only waits ~one chunk per ring
                xt_sb = xt_pool.tile([128, NC, L], BF)
                wv_sb = w1.tile([128, NC, H], BF, tag="w")
                for c in range(NC):
                    eng_w = nc.sync if c % 2 == 0 else nc.scalar
                    eng_x = nc.scalar if c % 2 == 0 else nc.sync
                    eng_w.dma_start(
                        wv_sb[:, c, :],
                        wvt_e[:].rearrange("(c p) d -> p c d", p=128)[:, c, :],
                    )
                    eng_x.dma_start(
                        xt_sb[:, c, :],
                        xt_e[:].rearrange("(c p) q -> p c q", p=128)[:, c, :],
                    )
                load_smalls()
                for lc in range(LC):
                    ps = psA.tile([128, 1024], F32, tag="psA")
                    for off, width in ((0, 512), (512, 256)):
                        for kc in range(NC):
                            nc.tensor.matmul(
                                ps[:, off : off + width],
                                xt_sb[:, kc, lc * 128 : lc * 128 + 128],
                                wv_sb[:, kc, off : off + width],
                                start=(kc == 0),
                                stop=(kc == NC - 1),
                            )
                    # evacuate + bv add (bias varies along free dim)
                    nc.vector.tensor_tensor(
                        v_sb[:, lc, :, 0:HD],
                        ps[:, 0:H].rearrange("p (h d) -> p h d", d=HD),
                        bv_sb[:].rearrange("p (h d) -> p h d", d=HD),
                        mybir.AluOpType.add,
                    )

                # ---- K^T then Q^T projections: out[d, q] = W x^T + b
                for w_e, b_sb, dst in ((wkt_e, bk_sb, kt_sb), (wqt_e, bq_sb, qt_sb)):
                    w_sb = w1.tile([128, NC, H], BF, tag="w")
                    nc.sync.dma_start(
                        w_sb[:], w_e[:].rearrange("(c p) d -> p c d", p=128)
                    )
                    for dc in range(NC):
                        ps = psA.tile([128, 1024], F32, tag="psA")
                        for qh in range(2):
                            o = qh * 512
                            for kc in range(NC):
                                nc.tensor.matmul(
                                    ps[:, o : o + 512],
                                    w_sb[:, kc, dc * 128 : dc * 128 + 128],
                                    xt_sb[:, kc, o : o + 512],
                                    start=(kc == 0),
                                    stop=(kc == NC - 1),
                                )
                        if dst is qt_sb:
                            nc.vector.tensor_scalar_add(
                                dst[:, dc, :], ps[:, :], b_sb[:, dc : dc + 1]
                            )
                        else:
                            nc.vector.tensor_scalar_add(
                                kt_sb[0:64, 2 * dc, :], ps[0:64, :],
                                b_sb[0:64, dc : dc + 1],
                            )
                            nc.vector.tensor_scalar_add(
                                kt_sb[64:128, 2 * dc + 1, :], ps[64:128, :],
                                b_sb[64:128, dc : dc + 1],
                            )

            # =========== attention + output projection ===========
            with (
                tc.tile_pool(name="w2", bufs=1) as w2,
                tc.tile_pool(name="et", bufs=6) as et_pool,
                tc.tile_pool(name="norm", bufs=2) as norm_pool,
            ):
                wo_sb = w2.tile([128, NC, H], BF)
                nc.sync.dma_start(
                    wo_sb[:], wot_e[:].rearrange("(c p) d -> p c d", p=128)
                )

                def emit_normalize(hp, ctxu):
                    # softmax 1/denom, entirely off the PE/ACT engines:
                    # pack both heads' denominator rows across 128 partitions
                    # (q = p*8+i per head), one cheap DVE reciprocal, then a
                    # DRAM roundtrip to replicate the reciprocals to 64 rows
                    # (engines can't broadcast across partitions; DMA can't
                    # use a 0-stride SBUF source, but a DRAM source works).
                    dpk = norm_pool.tile([128, 16], F32, tag="dpk")
                    nc.sync.dma_start(
                        dpk[:, 0:8],
                        ctxu[64:65, 0:1024].rearrange("o (p i) -> o p i", p=128),
                    )
                    nc.sync.dma_start(
                        dpk[:, 8:16],
                        ctxu[64:65, 1024:2048].rearrange("o (p i) -> o p i", p=128),
                    )
                    rpk = norm_pool.tile([128, 16], F32, tag="rpk")
                    nc.vector.reciprocal(rpk[:], dpk[:])
                    nc.sync.dma_start(
                        rscr_e[:].rearrange("(p x) -> p x", p=128), rpk[:]
                    )
                    bc = norm_pool.tile([64, 2048], F32, tag="bc")
                    rscr_pi = rscr_e[:].rearrange("(p x) -> p x", p=128)
                    nc.sync.dma_start(
                        bc[:, 0:1024].rearrange("d (p i) -> d p i", p=128),
                        rscr_pi[None, :, 0:8].to_broadcast((64, 128, 8)),
                    )
                    nc.sync.dma_start(
                        bc[:, 1024:2048].rearrange("d (p i) -> d p i", p=128),
                        rscr_pi[None, :, 8:16].to_broadcast((64, 128, 8)),
                    )
                    nc.vector.tensor_tensor(
                        ctxt_sb[0:64, hp, :], ctxu[0:64, 0:1024], bc[:, 0:1024],
                        mybir.AluOpType.mult,
                    )
                    tmp_o = norm_pool.tile([64, 1024], BF, tag="tmp")
                    nc.gpsimd.tensor_tensor(
                        tmp_o[:], ctxu[0:64, 1024:2048], bc[:, 1024:2048],
                        mybir.AluOpType.mult,
                    )
                    # lift odd head to partitions 64:128 (DMA can cross
                    # partitions; DVE cannot)
                    nc.sync.dma_start(ctxt_sb[64:128, hp, :], tmp_o[:])

                for hp in range(NH // 2):
                    ha, hb = 2 * hp, 2 * hp + 1
                    # per head: rows 0:64 = ctx^T, row 64 = softmax denominator
                    ctx_a = psB.tile([128, 1024], F32, tag="psB")
                    ctx_b = psB.tile([128, 1024], F32, tag="psB")

                    def emit_pv(kc, et_a, et_b, ctx_a=ctx_a, ctx_b=ctx_b,
                                ha=ha, hb=hb):
                        first, last = kc == 0, kc == LC - 1
                        for qh in range(2):
                            o = qh * 512
                            # ctx^T[d, q] += V^T P^T ; row 64 = denominator
                            nc.tensor.matmul(
                                ctx_a[0 : HD + 1, o : o + 512],
                                v_sb[:, kc, ha, :],
                                et_a[:, o : o + 512],
                                start=first,
                                stop=last,
                            )
                            nc.tensor.matmul(
                                ctx_b[0 : HD + 1, o : o + 512],
                                v_sb[:, kc, hb, :],
                                et_b[:, o : o + 512],
                                start=first,
                                stop=last,
                            )

                    pv_q = []  # software pipeline: PV(kc-1) after ST(kc)
                    for kc in range(LC):
                        # per-head ST then its exp immediately, so ScalarE
                        # starts head a's exp while the PE streams head b
                        st_a = psA.tile([128, 1024], F32, tag="psA")
                        st_b = psA.tile([128, 1024], F32, tag="psA")
                        for o in (0, 512):
                            nc.tensor.matmul(
                                st_a[:, o : o + 512],
                                kt_sb[:, ha, kc * 128 : kc * 128 + 128],
                                qt_sb[:, hp, o : o + 512],
                                start=True,
                                stop=True,
                            )
                        et_a = et_pool.tile([128, 1024], BF, tag="et")
                        nc.scalar.activation(
                            et_a[:], st_a[:], EXP,
                            bias=mask_sb[:, kc : kc + 1], scale=0.125,
                        )
                        for o in (0, 512):
                            nc.tensor.matmul(
                                st_b[:, o : o + 512],
                                kt_sb[:, hb, kc * 128 : kc * 128 + 128],
                                qt_sb[:, hp, o : o + 512],
                                start=True,
                                stop=True,
                            )
                        et_b = et_pool.tile([128, 1024], BF, tag="et")
                        nc.scalar.activation(
                            et_b[:], st_b[:], EXP,
                            bias=mask_sb[:, kc : kc + 1], scale=0.125,
                        )
                        pv_q.append((kc, et_a, et_b))
                        if kc >= 1:
                            emit_pv(*pv_q.pop(0))
                    emit_pv(*pv_q.pop(0))
                    # evacuate ctx+denominator to SBUF immediately: frees the
                    # PSUM slots for the next pair.  Head a evacuates on DVE,
                    # head b on ScalarE (activation Copy; GpSimd can't read
                    # PSUM per the BIR verifier), so the two copies run in
                    # parallel and neither queues behind the previous pair's
                    # normalize chain.
                    ctxu = norm_pool.tile([65, 2048], F32, tag="cu")
                    nc.vector.tensor_copy(ctxu[:, 0:1024], ctx_a[0:65, :])
                    nc.scalar.activation(
                        ctxu[:, 1024:2048], ctx_b[0:65, :],
                        mybir.ActivationFunctionType.Copy,
                    )
                    emit_normalize(hp, ctxu)

                # ---- output projection: out[q, o] = ctx Wo^T + bo ----
                # Emitted split: chunks 0..4 of up to four lc groups stream
                # first (psA+psB pools, 4 open accumulation groups), the
                # chunk-5 matmuls (gated on the last head pair's normalize
                # chain) trail behind — so the PE keeps streaming while the
                # final normalize's DMA ladder completes.
                ps_of = {}

                def op_partial(lc):
                    pool = psA if (lc % 2 == 0) else psB
                    ps = pool.tile([128, 1024], F32, tag=pool is psA and "psA" or "psB")
                    ps_of[lc] = ps
                    for off, width in ((0, 512), (512, 256)):
                        for c in range(NC - 1):
                            nc.tensor.matmul(
                                ps[:, off : off + width],
                                ctxt_sb[:, c, lc * 128 : lc * 128 + 128],
                                wo_sb[:, c, off : off + width],
                                start=(c == 0),
                                stop=False,
                            )

                def op_finish(lc):
                    ps = ps_of.pop(lc)
                    c = NC - 1
                    for off, width in ((0, 512), (512, 256)):
                        nc.tensor.matmul(
                            ps[:, off : off + width],
                            ctxt_sb[:, c, lc * 128 : lc * 128 + 128],
                            wo_sb[:, c, off : off + width],
                            start=False,
                            stop=True,
                        )
                    o_sb = out_pool.tile([128, H], F32, tag="outp")
                    nc.vector.tensor_tensor(
                        o_sb[:], ps[:, 0:H], bo_sb[:], mybir.AluOpType.add
                    )
                    nc.sync.dma_start(out_e[lc * 128 : lc * 128 + 128, :], o_sb[:])

                for lc in range(4):
                    op_partial(lc)
                for lc in range(4, LC):
                    op_finish(lc - 4)
                    op_partial(lc)
                for lc in range(LC - 4, LC):
                    op_finish(lc)

    nc.finalize()
    nc.m = get_hw_module(nc.m)
    return nc


_NC_CACHE = {}


def _get_nc(compute_rounded: bool = True):
    if compute_rounded not in _NC_CACHE:
        _NC_CACHE[compute_rounded] = build_bass(compute_rounded)
    return _NC_CACHE[compute_rounded]


def make_in_maps(inputs):
    f = lambda a: np.ascontiguousarray(np.asarray(a, dtype=np.float32))  # noqa: E731
    fb = lambda a: np.ascontiguousarray(  # noqa: E731
        np.asarray(a, dtype=np.float32).astype(ml_dtypes.bfloat16)
    )
    hs = f(inputs["hidden_states"])
    mask = f(inputs["attention_mask"]).reshape(B, L)
    shared = {
        "wqt": fb(np.asarray(inputs["Wq"]).T),
        "wkt": fb(np.asarray(inputs["Wk"]).T),
        "wvt": fb(np.asarray(inputs["Wv"]).T),
        "wot": fb(np.asarray(inputs["Wo"]).T),
        "bq": f(inputs["bq"]),
        "bk": f(inputs["bk"]),
        "bv": f(inputs["bv"]),
        "bo": f(inputs["bo"]),
    }
    return [
        {"xt": fb(hs[b].T), "mask": mask[b], **shared}
        for b in range(B)
    ]


def run_spmd(inputs, trace=False, compute_rounded=True):
    nc = _get_nc(compute_rounded)
    res = run_bass_kernel_spmd(nc, make_in_maps(inputs), list(range(B)), trace=trace)
    out = np.stack([res.results[b]["out"] for b in range(B)]).astype(np.float32)
    return out, res


def kernel(**inputs) -> np.ndarray:
    out, _ = run_spmd(inputs, trace=False)
    return out


# revision 15
# speedup vs baseline: 1.4596x; 1.0936x over previous
"""BERT self-attention on 8 Trainium2 NeuronCores.

Sharding: data-parallel over batch (B=8 -> one batch element per core).
Each core computes full self-attention for its batch element:
  Q/K/V projections, per-head softmax(Q K^T / 8 + mask) V, output proj.

Layout strategy (per core):
  - Host passes xt = x.T [768,1024] and W.T [768,768] in bf16 so every
    matmul contracts over the partition axis and streams at 1 cyc/row.
  - QT,KT [d, L] and V [L, d] are produced directly by the projections.
  - Attention runs transposed: ST[k,q] = K Q^T per head, so softmax's
    reduction axis (k) lands on partitions: exp via ScalarE with the
    attention mask as per-partition bias (no max subtraction: scores are
    ~N(0,1), |s|<~7, exp is safe in fp32); the denominator comes from a
    ones column appended to V (out row 64); P^T V accumulates ctx^T
    [d, q] which feeds the output projection as lhsT directly.
  - All matmul inputs are bf16 (fp32 PSUM accumulation); measured rel
    err vs the fp32 reference is ~7e-3.
  - The PE does only the structural matmuls: QKV/out biases are folded
    into the PSUM->SBUF evacuation ops against DMA-broadcast bias rows,
    and the softmax 1/denom broadcast is computed entirely off the PE:
    the two denominator rows are DMA-packed across 128 partitions
    ([128,16]), reciprocated in one cheap DVE op, DMA-unpacked, and
    partition-broadcast on GpSimd.  This keeps ScalarE exp-only, which
    is what paces the attention inner loop.
  - Startup: weight chunks load on the Sync HWDGE ring while xt chunks
    load on the Scalar HWDGE ring, halving time-to-first-matmul.
"""

import numpy as np
import ml_dtypes

import concourse.bass as bass  # noqa: F401
import concourse.mybir as mybir
import concourse.tile as tile
from concourse import bacc
from concourse.bass_interp import get_hw_module
from concourse.bass_utils import run_bass_kernel_spmd

B, L, H = 8, 1024, 768
NH, HD = 12, 64
NC = H // 128          # 6 chunks of hidden dim
LC = L // 128          # 8 chunks of sequence dim
F32 = mybir.dt.float32
BF = mybir.dt.bfloat16
EXP = mybir.ActivationFunctionType.Exp


def build_bass(compute_rounded: bool = True):
    del compute_rounded  # single all-bf16 variant

    nc = bacc.Bacc("TRN2", debug=False, num_devices=8)

    xt_e = nc.declare_dram_parameter("xt", [H, L], BF, isOutput=False)
    wqt_e = nc.declare_dram_parameter("wqt", [H, H], BF, isOutput=False)
    wkt_e = nc.declare_dram_parameter("wkt", [H, H], BF, isOutput=False)
    wvt_e = nc.declare_dram_parameter("wvt", [H, H], BF, isOutput=False)
    wot_e = nc.declare_dram_parameter("wot", [H, H], BF, isOutput=False)
    bq_e = nc.declare_dram_parameter("bq", [H], F32, isOutput=False)
    bk_e = nc.declare_dram_parameter("bk", [H], F32, isOutput=False)
    bv_e = nc.declare_dram_parameter("bv", [H], F32, isOutput=False)
    bo_e = nc.declare_dram_parameter("bo", [H], F32, isOutput=False)
    mask_e = nc.declare_dram_parameter("mask", [L], F32, isOutput=False)
    out_e = nc.declare_dram_parameter("out", [L, H], F32, isOutput=True)
    # DRAM scratch for the softmax reciprocal broadcast roundtrip
    rscr_e = nc.dram_tensor("rscr", [2048], F32)

    with tile.TileContext(nc) as tc:
        with (
            tc.tile_pool(name="small", bufs=1) as small,
            tc.tile_pool(name="acts", bufs=1) as acts,
            tc.tile_pool(name="outp", bufs=2) as out_pool,
            tc.tile_pool(name="psA", bufs=2, space="PSUM") as psA,
            tc.tile_pool(name="psB", bufs=2, space="PSUM") as psB,
        ):
            # ---- constants / small tensors (DMAs issued after wv/xt: the
            # V-proj's first matmul gates on wv/xt chunk 0, these don't) ----
            mask_sb = small.tile([128, LC], F32)
            bq_sb = small.tile([128, NC], F32)
            bk_sb = small.tile([128, NC], F32)
            bv_sb = small.tile([128, H], F32)
            bo_sb = small.tile([128, H], F32)

            def load_smalls():
                nc.sync.dma_start(
                    mask_sb[:], mask_e[:].rearrange("(c p) -> p c", p=128)
                )
                nc.sync.dma_start(bq_sb[:], bq_e[:].rearrange("(c p) -> p c", p=128))
                nc.sync.dma_start(bk_sb[:], bk_e[:].rearrange("(c p) -> p c", p=128))
                # bias rows replicated across all partitions (free-dim biases)
                nc.sync.dma_start(bv_sb[:], bv_e[None, :].to_broadcast((128, H)))
                nc.sync.dma_start(bo_sb[:], bo_e[None, :].to_broadcast((128, H)))

            qt_sb = acts.tile([128, NC, L], BF)
            kt_sb = acts.tile([128, NH, L], BF)  # per-head K^T, other 64 rows zero
            nc.gpsimd.memset(kt_sb[:], 0.0)
            v_sb = acts.tile([128, LC, NH, HD + 1], BF)  # [..., 64] = ones col
            ctxt_sb = acts.tile([128, NC, L], BF)
            nc.vector.memset(v_sb[:, :, :, HD], 1.0)

            # =========== projection phase (xt + wv/wk/wq scoped) ===========
            with (
                tc.tile_pool(name="xt", bufs=1) as xt_pool,
                tc.tile_pool(name="w1", bufs=2) as w1,
            ):
                # weights ride the Sync HWDGE ring, xt the Scalar ring, so
                # the V-proj's first matmul only waits ~one chunk per ring
                xt_sb = xt_pool.tile([128, NC, L], BF)
                wv_sb = w1.tile([128, NC, H], BF, tag="w")
                for c in range(NC):
                    eng_w = nc.sync if c % 2 == 0 else nc.scalar
                    eng_x = nc.scalar if c % 2 == 0 else nc.sync
                    eng_w.dma_start(
                        wv_sb[:, c, :],
                        wvt_e[:].rearrange("(c p) d -> p c d", p=128)[:, c, :],
                    )
                    eng_x.dma_start(
                        xt_sb[:, c, :],
                        xt_e[:].rearrange("(c p) q -> p c q", p=128)[:, c, :],
                    )
                load_smalls()
                for lc in range(LC):
                    ps = psA.tile([128, 1024], F32, tag="psA")
                    for off, width in ((0, 512), (512, 256)):
                        for kc in range(NC):
                            nc.tensor.matmul(
                                ps[:, off : off + width],
                                xt_sb[:, kc, lc * 128 : lc * 128 + 128],
                                wv_sb[:, kc, off : off + width],
                                start=(kc == 0),
                                stop=(kc == NC - 1),
                            )
                    # evacuate + bv add (bias varies along free dim)
                    nc.vector.tensor_tensor(
                        v_sb[:, lc, :, 0:HD],
                        ps[:, 0:H].rearrange("p (h d) -> p h d", d=HD),
                        bv_sb[:].rearrange("p (h d) -> p h d", d=HD),
                        mybir.AluOpType.add,
                    )

                # ---- K^T then Q^T projections: out[d, q] = W x^T + b
                for w_e, b_sb, dst in ((wkt_e, bk_sb, kt_sb), (wqt_e, bq_sb, qt_sb)):
                    w_sb = w1.tile([128, NC, H], BF, tag="w")
                    nc.sync.dma_start(
                        w_sb[:], w_e[:].rearrange("(c p) d -> p c d", p=128)
                    )
                    for dc in range(NC):
                        ps = psA.tile([128, 1024], F32, tag="psA")
                        for qh in range(2):
                            o = qh * 512
                            for kc in range(NC):
                                nc.tensor.matmul(
                                    ps[:, o : o + 512],
                                    w_sb[:, kc, dc * 128 : dc * 128 + 128],
                                    xt_sb[:, kc, o : o + 512],
                                    start=(kc == 0),
                                    stop=(kc == NC - 1),
                                )
                        if dst is qt_sb:
                            nc.vector.tensor_scalar_add(
                                dst[:, dc, :], ps[:, :], b_sb[:, dc : dc + 1]
                            )
                        else:
                            nc.vector.tensor_scalar_add(
                                kt_sb[0:64, 2 * dc, :], ps[0:64, :],
                                b_sb[0:64, dc : dc + 1],
                            )
                            nc.vector.tensor_scalar_add(
                                kt_sb[64:128, 2 * dc + 1, :], ps[64:128, :],
                                b_sb[64:128, dc : dc + 1],
                            )

            # =========== attention + output projection ===========
            with (
                tc.tile_pool(name="w2", bufs=1) as w2,
                tc.tile_pool(name="et", bufs=6) as et_pool,
                tc.tile_pool(name="norm", bufs=2) as norm_pool,
            ):
                wo_sb = w2.tile([128, NC, H], BF)
                nc.sync.dma_start(
                    wo_sb[:], wot_e[:].rearrange("(c p) d -> p c d", p=128)
                )

                def emit_normalize(hp, ctxu):
                    # softmax 1/denom, entirely off the PE/ACT engines:
                    # pack both heads' denominator rows across 128 partitions
                    # (q = p*8+i per head), one cheap DVE reciprocal, then a
                    # DRAM roundtrip to replicate the reciprocals to 64 rows
                    # (engines can't broadcast across partitions; DMA can't
                    # use a 0-stride SBUF source, but a DRAM source works).
                    dpk = norm_pool.tile([128, 16], F32, tag="dpk")
                    nc.sync.dma_start(
                        dpk[:, 0:8],
                        ctxu[64:65, 0:1024].rearrange("o (p i) -> o p i", p=128),
                    )
                    nc.sync.dma_start(
                        dpk[:, 8:16],
                        ctxu[64:65, 1024:2048].rearrange("o (p i) -> o p i", p=128),
                    )
                    rpk = norm_pool.tile([128, 16], F32, tag="rpk")
                    nc.vector.reciprocal(rpk[:], dpk[:])
                    # scatter-write the 8KB of reciprocals into q-order in
                    # DRAM, so the two 256KB broadcast reads below stream
                    # fully contiguous 4KB rows (a 32B-granule broadcast
                    # read measured ~5us; this shape is ~1us)
                    nc.sync.dma_start(
                        rscr_e[:].rearrange("(h p i) -> p h i", h=2, p=128),
                        rpk[:].rearrange("p (h i) -> p h i", h=2),
                    )
                    bc = norm_pool.tile([64, 2048], F32, tag="bc")
                    rscr_hq = rscr_e[:].rearrange("(h q) -> h q", h=2)
                    nc.sync.dma_start(
                        bc[:, 0:1024], rscr_hq[0:1, :].to_broadcast((64, 1024))
                    )
                    nc.sync.dma_start(
                        bc[:, 1024:2048], rscr_hq[1:2, :].to_broadcast((64, 1024))
                    )
                    nc.vector.tensor_tensor(
                        ctxt_sb[0:64, hp, :], ctxu[0:64, 0:1024], bc[:, 0:1024],
                        mybir.AluOpType.mult,
                    )
                    tmp_o = norm_pool.tile([64, 1024], BF, tag="tmp")
                    nc.gpsimd.tensor_tensor(
                        tmp_o[:], ctxu[0:64, 1024:2048], bc[:, 1024:2048],
                        mybir.AluOpType.mult,
                    )
                    # lift odd head to partitions 64:128 (DMA can cross
                    # partitions; DVE cannot)
                    nc.sync.dma_start(ctxt_sb[64:128, hp, :], tmp_o[:])

                for hp in range(NH // 2):
                    ha, hb = 2 * hp, 2 * hp + 1
                    # per head: rows 0:64 = ctx^T, row 64 = softmax denominator
                    ctx_a = psB.tile([128, 1024], F32, tag="psB")
                    ctx_b = psB.tile([128, 1024], F32, tag="psB")

                    def emit_pv(kc, et_a, et_b, ctx_a=ctx_a, ctx_b=ctx_b,
                                ha=ha, hb=hb):
                        first, last = kc == 0, kc == LC - 1
                        for qh in range(2):
                            o = qh * 512
                            # ctx^T[d, q] += V^T P^T ; row 64 = denominator
                            nc.tensor.matmul(
                                ctx_a[0 : HD + 1, o : o + 512],
                                v_sb[:, kc, ha, :],
                                et_a[:, o : o + 512],
                                start=first,
                                stop=last,
                            )
                            nc.tensor.matmul(
                                ctx_b[0 : HD + 1, o : o + 512],
                                v_sb[:, kc, hb, :],
                                et_b[:, o : o + 512],
                                start=first,
                                stop=last,
                            )

                    pv_q = []  # software pipeline: PV(kc-1) after ST(kc)
                    for kc in range(LC):
                        # per-head ST then its exp immediately, so ScalarE
                        # starts head a's exp while the PE streams head b
                        st_a = psA.tile([128, 1024], F32, tag="psA")
                        st_b = psA.tile([128, 1024], F32, tag="psA")
                        for o in (0, 512):
                            nc.tensor.matmul(
                                st_a[:, o : o + 512],
                                kt_sb[:, ha, kc * 128 : kc * 128 + 128],
                                qt_sb[:, hp, o : o + 512],
                                start=True,
                                stop=True,
                            )
                        et_a = et_pool.tile([128, 1024], BF, tag="et")
                        nc.scalar.activation(
                            et_a[:], st_a[:], EXP,
                            bias=mask_sb[:, kc : kc + 1], scale=0.125,
                        )
                        for o in (0, 512):
                            nc.tensor.matmul(
                                st_b[:, o : o + 512],
                                kt_sb[:, hb, kc * 128 : kc * 128 + 128],
                                qt_sb[:, hp, o : o + 512],
                                start=True,
                                stop=True,
                            )
                        et_b = et_pool.tile([128, 1024], BF, tag="et")
                        nc.scalar.activation(
                            et_b[:], st_b[:], EXP,
                            bias=mask_sb[:, kc : kc + 1], scale=0.125,
                        )
                        pv_q.append((kc, et_a, et_b))
                        if kc >= 1:
                            emit_pv(*pv_q.pop(0))
                    emit_pv(*pv_q.pop(0))
                    # evacuate ctx+denominator to SBUF immediately: frees the
                    # PSUM slots for the next pair.  Head a evacuates on DVE,
                    # head b on ScalarE (activation Copy; GpSimd can't read
                    # PSUM per the BIR verifier), so the two copies run in
                    # parallel and neither queues behind the previous pair's
                    # normalize chain.
                    ctxu = norm_pool.tile([65, 2048], F32, tag="cu")
                    nc.vector.tensor_copy(ctxu[:, 0:1024], ctx_a[0:65, :])
                    nc.scalar.activation(
                        ctxu[:, 1024:2048], ctx_b[0:65, :],
                        mybir.ActivationFunctionType.Copy,
                    )
                    emit_normalize(hp, ctxu)

                # ---- output projection: out[q, o] = ctx Wo^T + bo ----
                # Emitted split: chunks 0..4 of up to four lc groups stream
                # first (psA+psB pools, 4 open accumulation groups), the
                # chunk-5 matmuls (gated on the last head pair's normalize
                # chain) trail behind — so the PE keeps streaming while the
                # final normalize's DMA ladder completes.
                ps_of = {}

                def op_partial(lc):
                    pool = psA if (lc % 2 == 0) else psB
                    ps = pool.tile([128, 1024], F32, tag=pool is psA and "psA" or "psB")
                    ps_of[lc] = ps
                    for off, width in ((0, 512), (512, 256)):
                        for c in range(NC - 1):
                            nc.tensor.matmul(
                                ps[:, off : off + width],
                                ctxt_sb[:, c, lc * 128 : lc * 128 + 128],
                                wo_sb[:, c, off : off + width],
                                start=(c == 0),
                                stop=False,
                            )

                def op_finish(lc):
                    ps = ps_of.pop(lc)
                    c = NC - 1
                    for off, width in ((0, 512), (512, 256)):
                        nc.tensor.matmul(
                            ps[:, off : off + width],
                            ctxt_sb[:, c, lc * 128 : lc * 128 + 128],
                            wo_sb[:, c, off : off + width],
                            start=False,
                            stop=True,
                        )
                    o_sb = out_pool.tile([128, H], F32, tag="outp")
                    nc.vector.tensor_tensor(
                        o_sb[:], ps[:, 0:H], bo_sb[:], mybir.AluOpType.add
                    )
                    nc.sync.dma_start(out_e[lc * 128 : lc * 128 + 128, :], o_sb[:])

                for lc in range(4):
                    op_partial(lc)
                for lc in range(4, LC):
                    op_finish(lc - 4)
                    op_partial(lc)
                for lc in range(LC - 4, LC):
                    op_finish(lc)

    nc.finalize()
    nc.m = get_hw_module(nc.m)
    return nc


_NC_CACHE = {}


def _get_nc(compute_rounded: bool = True):
    if compute_rounded not in _NC_CACHE:
        _NC_CACHE[compute_rounded] = build_bass(compute_rounded)
    return _NC_CACHE[compute_rounded]


def make_in_maps(inputs):
    f = lambda a: np.ascontiguousarray(np.asarray(a, dtype=np.float32))  # noqa: E731
    fb = lambda a: np.ascontiguousarray(  # noqa: E731
        np.asarray(a, dtype=np.float32).astype(ml_dtypes.bfloat16)
    )
    hs = f(inputs["hidden_states"])
    mask = f(inputs["attention_mask"]).reshape(B, L)
    shared = {
        "wqt": fb(np.asarray(inputs["Wq"]).T),
        "wkt": fb(np.asarray(inputs["Wk"]).T),
        "wvt": fb(np.asarray(inputs["Wv"]).T),
        "wot": fb(np.asarray(inputs["Wo"]).T),
        "bq": f(inputs["bq"]),
        "bk": f(inputs["bk"]),
        "bv": f(inputs["bv"]),
        "bo": f(inputs["bo"]),
    }
    return [
        {"xt": fb(hs[b].T), "mask": mask[b], **shared}
        for b in range(B)
    ]


def run_spmd(inputs, trace=False, compute_rounded=True):
    nc = _get_nc(compute_rounded)
    res = run_bass_kernel_spmd(nc, make_in_maps(inputs), list(range(B)), trace=trace)
    out = np.stack([res.results[b]["out"] for b in range(B)]).astype(np.float32)
    return out, res


def kernel(**inputs) -> np.ndarray:
    out, _ = run_spmd(inputs, trace=False)
    return out


# revision 16
# speedup vs baseline: 1.5474x; 1.0602x over previous
"""BERT self-attention on 8 Trainium2 NeuronCores.

Sharding: data-parallel over batch (B=8 -> one batch element per core).
Each core computes full self-attention for its batch element:
  Q/K/V projections, per-head softmax(Q K^T / 8 + mask) V, output proj.

Layout strategy (per core):
  - Host passes xt = x.T [768,1024] and W.T [768,768] in bf16 so every
    matmul contracts over the partition axis and streams at 1 cyc/row.
  - QT,KT [d, L] and V [L, d] are produced directly by the projections.
  - Attention runs transposed: ST[k,q] = K Q^T per head, so softmax's
    reduction axis (k) lands on partitions: exp via ScalarE with the
    attention mask as per-partition bias (no max subtraction: scores are
    ~N(0,1), |s|<~7, exp is safe in fp32); the denominator comes from a
    ones column appended to V (out row 64); P^T V accumulates ctx^T
    [d, q] which feeds the output projection as lhsT directly.
  - All matmul inputs are bf16 (fp32 PSUM accumulation); measured rel
    err vs the fp32 reference is ~7e-3.
  - The PE does only the structural matmuls: QKV/out biases are folded
    into the PSUM->SBUF evacuation ops against DMA-broadcast bias rows,
    and the softmax 1/denom broadcast is computed entirely off the PE:
    the two denominator rows are DMA-packed across 128 partitions
    ([128,16]), reciprocated in one cheap DVE op, DMA-unpacked, and
    partition-broadcast on GpSimd.  This keeps ScalarE exp-only, which
    is what paces the attention inner loop.
  - Startup: weight chunks load on the Sync HWDGE ring while xt chunks
    load on the Scalar HWDGE ring, halving time-to-first-matmul.
"""

import numpy as np
import ml_dtypes

import concourse.bass as bass  # noqa: F401
import concourse.mybir as mybir
import concourse.tile as tile
from concourse import bacc
from concourse.bass_interp import get_hw_module
from concourse.bass_utils import run_bass_kernel_spmd

B, L, H = 8, 1024, 768
NH, HD = 12, 64
NC = H // 128          # 6 chunks of hidden dim
LC = L // 128          # 8 chunks of sequence dim
F32 = mybir.dt.float32
BF = mybir.dt.bfloat16
EXP = mybir.ActivationFunctionType.Exp


def build_bass(compute_rounded: bool = True):
    del compute_rounded  # single all-bf16 variant

    nc = bacc.Bacc("TRN2", debug=False, num_devices=8)

    xt_e = nc.declare_dram_parameter("xt", [H, L], BF, isOutput=False)
    wqt_e = nc.declare_dram_parameter("wqt", [H, H], BF, isOutput=False)
    wkt_e = nc.declare_dram_parameter("wkt", [H, H], BF, isOutput=False)
    wvt_e = nc.declare_dram_parameter("wvt", [H, H], BF, isOutput=False)
    wot_e = nc.declare_dram_parameter("wot", [H, H], BF, isOutput=False)
    bq_e = nc.declare_dram_parameter("bq", [H], F32, isOutput=False)
    bk_e = nc.declare_dram_parameter("bk", [H], F32, isOutput=False)
    bv_e = nc.declare_dram_parameter("bv", [H], F32, isOutput=False)
    bo_e = nc.declare_dram_parameter("bo", [H], F32, isOutput=False)
    mask_e = nc.declare_dram_parameter("mask", [L], F32, isOutput=False)
    out_e = nc.declare_dram_parameter("out", [L, H], F32, isOutput=True)
    # DRAM scratch for the softmax reciprocal broadcast roundtrip
    rscr_e = nc.dram_tensor("rscr", [2048], F32)

    with tile.TileContext(nc) as tc:
        with (
            tc.tile_pool(name="small", bufs=1) as small,
            tc.tile_pool(name="acts", bufs=1) as acts,
            tc.tile_pool(name="outp", bufs=4) as out_pool,
            tc.tile_pool(name="psA", bufs=2, space="PSUM") as psA,
            tc.tile_pool(name="psB", bufs=2, space="PSUM") as psB,
        ):
            # ---- constants / small tensors (DMAs issued after wv/xt: the
            # V-proj's first matmul gates on wv/xt chunk 0, these don't) ----
            mask_sb = small.tile([128, LC], F32)
            bq_sb = small.tile([128, NC], F32)
            bk_sb = small.tile([128, NC], F32)
            bv_sb = small.tile([128, H], F32)
            bo_sb = small.tile([128, H], F32)

            def load_smalls():
                nc.sync.dma_start(
                    mask_sb[:], mask_e[:].rearrange("(c p) -> p c", p=128)
                )
                nc.sync.dma_start(bq_sb[:], bq_e[:].rearrange("(c p) -> p c", p=128))
                nc.sync.dma_start(bk_sb[:], bk_e[:].rearrange("(c p) -> p c", p=128))
                # bias rows replicated across all partitions (free-dim biases)
                nc.sync.dma_start(bv_sb[:], bv_e[None, :].to_broadcast((128, H)))
                nc.sync.dma_start(bo_sb[:], bo_e[None, :].to_broadcast((128, H)))

            qt_sb = acts.tile([128, NC, L], BF)
            kt_sb = acts.tile([128, NH, L], BF)  # per-head K^T, other 64 rows zero
            nc.gpsimd.memset(kt_sb[:], 0.0)
            v_sb = acts.tile([128, LC, NH, HD + 1], BF)  # [..., 64] = ones col
            ctxt_sb = acts.tile([128, NC, L], BF)
            nc.vector.memset(v_sb[:, :, :, HD], 1.0)

            # =========== projection phase (xt + wv/wk/wq scoped) ===========
            with (
                tc.tile_pool(name="xt", bufs=1) as xt_pool,
                tc.tile_pool(name="w1", bufs=2) as w1,
            ):
                # weights ride the Sync HWDGE ring, xt the Scalar ring, so
                # the V-proj's first matmul only waits ~one chunk per ring
                xt_sb = xt_pool.tile([128, NC, L], BF)
                wv_sb = w1.tile([128, NC, H], BF, tag="w")
                for c in range(NC):
                    eng_w = nc.sync if c % 2 == 0 else nc.scalar
                    eng_x = nc.scalar if c % 2 == 0 else nc.sync
                    eng_w.dma_start(
                        wv_sb[:, c, :],
                        wvt_e[:].rearrange("(c p) d -> p c d", p=128)[:, c, :],
                    )
                    eng_x.dma_start(
                        xt_sb[:, c, :],
                        xt_e[:].rearrange("(c p) q -> p c q", p=128)[:, c, :],
                    )
                load_smalls()
                for lc in range(LC):
                    pool = psA if lc % 2 == 0 else psB
                    ps = pool.tile([128, 1024], F32, tag="psA" if lc % 2 == 0 else "psB")
                    for off, width in ((0, 512), (512, 256)):
                        for kc in range(NC):
                            nc.tensor.matmul(
                                ps[:, off : off + width],
                                xt_sb[:, kc, lc * 128 : lc * 128 + 128],
                                wv_sb[:, kc, off : off + width],
                                start=(kc == 0),
                                stop=(kc == NC - 1),
                            )
                    # evacuate + bv add (bias varies along free dim)
                    nc.vector.tensor_tensor(
                        v_sb[:, lc, :, 0:HD],
                        ps[:, 0:H].rearrange("p (h d) -> p h d", d=HD),
                        bv_sb[:].rearrange("p (h d) -> p h d", d=HD),
                        mybir.AluOpType.add,
                    )

                # ---- K^T then Q^T projections: out[d, q] = W x^T + b
                for w_e, b_sb, dst in ((wkt_e, bk_sb, kt_sb), (wqt_e, bq_sb, qt_sb)):
                    w_sb = w1.tile([128, NC, H], BF, tag="w")
                    nc.sync.dma_start(
                        w_sb[:], w_e[:].rearrange("(c p) d -> p c d", p=128)
                    )
                    for dc in range(NC):
                        pool = psA if dc % 2 == 0 else psB
                        ps = pool.tile(
                            [128, 1024], F32, tag="psA" if dc % 2 == 0 else "psB"
                        )
                        for qh in range(2):
                            o = qh * 512
                            for kc in range(NC):
                                nc.tensor.matmul(
                                    ps[:, o : o + 512],
                                    w_sb[:, kc, dc * 128 : dc * 128 + 128],
                                    xt_sb[:, kc, o : o + 512],
                                    start=(kc == 0),
                                    stop=(kc == NC - 1),
                                )
                        if dst is qt_sb:
                            nc.vector.tensor_scalar_add(
                                dst[:, dc, :], ps[:, :], b_sb[:, dc : dc + 1]
                            )
                        else:
                            nc.vector.tensor_scalar_add(
                                kt_sb[0:64, 2 * dc, :], ps[0:64, :],
                                b_sb[0:64, dc : dc + 1],
                            )
                            nc.vector.tensor_scalar_add(
                                kt_sb[64:128, 2 * dc + 1, :], ps[64:128, :],
                                b_sb[64:128, dc : dc + 1],
                            )

            # =========== attention + output projection ===========
            with (
                tc.tile_pool(name="w2", bufs=1) as w2,
                tc.tile_pool(name="et", bufs=6) as et_pool,
                tc.tile_pool(name="norm", bufs=2) as norm_pool,
            ):
                wo_sb = w2.tile([128, NC, H], BF)
                nc.sync.dma_start(
                    wo_sb[:], wot_e[:].rearrange("(c p) d -> p c d", p=128)
                )

                def emit_normalize(hp, ctxu, tail=False):
                    # softmax 1/denom, entirely off the PE/ACT engines:
                    # pack both heads' denominator rows across 128 partitions
                    # (q = p*8+i per head), one cheap DVE reciprocal, then a
                    # DRAM roundtrip to replicate the reciprocals to 64 rows
                    # (engines can't broadcast across partitions; DMA can't
                    # use a 0-stride SBUF source, but a DRAM source works).
                    dpk = norm_pool.tile([128, 16], F32, tag="dpk")
                    nc.sync.dma_start(
                        dpk[:, 0:8],
                        ctxu[64:65, 0:1024].rearrange("o (p i) -> o p i", p=128),
                    )
                    nc.sync.dma_start(
                        dpk[:, 8:16],
                        ctxu[64:65, 1024:2048].rearrange("o (p i) -> o p i", p=128),
                    )
                    rpk = norm_pool.tile([128, 16], F32, tag="rpk")
                    nc.vector.reciprocal(rpk[:], dpk[:])
                    # scatter-write the 8KB of reciprocals into q-order in
                    # DRAM, so the two 256KB broadcast reads below stream
                    # fully contiguous 4KB rows (a 32B-granule broadcast
                    # read measured ~5us; this shape is ~1us)
                    nc.sync.dma_start(
                        rscr_e[:].rearrange("(h p i) -> p h i", h=2, p=128),
                        rpk[:].rearrange("p (h i) -> p h i", h=2),
                    )
                    bc = norm_pool.tile([64, 2048], F32, tag="bc")
                    rscr_hq = rscr_e[:].rearrange("(h q) -> h q", h=2)
                    # at the tail ScalarE is done with exps, so its HWDGE
                    # ring can carry one broadcast in parallel
                    eng_b = nc.scalar if tail else nc.sync
                    nc.sync.dma_start(
                        bc[:, 0:1024], rscr_hq[0:1, :].to_broadcast((64, 1024))
                    )
                    eng_b.dma_start(
                        bc[:, 1024:2048], rscr_hq[1:2, :].to_broadcast((64, 1024))
                    )
                    nc.vector.tensor_tensor(
                        ctxt_sb[0:64, hp, :], ctxu[0:64, 0:1024], bc[:, 0:1024],
                        mybir.AluOpType.mult,
                    )
                    tmp_o = norm_pool.tile([64, 1024], BF, tag="tmp")
                    nc.vector.tensor_tensor(
                        tmp_o[:], ctxu[0:64, 1024:2048], bc[:, 1024:2048],
                        mybir.AluOpType.mult,
                    )
                    # lift odd head to partitions 64:128 (DMA can cross
                    # partitions; DVE cannot)
                    nc.sync.dma_start(ctxt_sb[64:128, hp, :], tmp_o[:])

                for hp in range(NH // 2):
                    ha, hb = 2 * hp, 2 * hp + 1
                    # per head: rows 0:64 = ctx^T, row 64 = softmax denominator
                    ctx_a = psB.tile([128, 1024], F32, tag="psB")
                    ctx_b = psB.tile([128, 1024], F32, tag="psB")

                    def emit_pv(kc, et_a, et_b, ctx_a=ctx_a, ctx_b=ctx_b,
                                ha=ha, hb=hb):
                        first, last = kc == 0, kc == LC - 1
                        for qh in range(2):
                            o = qh * 512
                            # ctx^T[d, q] += V^T P^T ; row 64 = denominator
                            nc.tensor.matmul(
                                ctx_a[0 : HD + 1, o : o + 512],
                                v_sb[:, kc, ha, :],
                                et_a[:, o : o + 512],
                                start=first,
                                stop=last,
                            )
                            nc.tensor.matmul(
                                ctx_b[0 : HD + 1, o : o + 512],
                                v_sb[:, kc, hb, :],
                                et_b[:, o : o + 512],
                                start=first,
                                stop=last,
                            )

                    pv_q = []  # software pipeline: PV(kc-1) after ST(kc)
                    for kc in range(LC):
                        # per-head ST then its exp immediately, so ScalarE
                        # starts head a's exp while the PE streams head b
                        st_a = psA.tile([128, 1024], F32, tag="psA")
                        st_b = psA.tile([128, 1024], F32, tag="psA")
                        for o in (0, 512):
                            nc.tensor.matmul(
                                st_a[:, o : o + 512],
                                kt_sb[:, ha, kc * 128 : kc * 128 + 128],
                                qt_sb[:, hp, o : o + 512],
                                start=True,
                                stop=True,
                            )
                        et_a = et_pool.tile([128, 1024], BF, tag="et")
                        nc.scalar.activation(
                            et_a[:], st_a[:], EXP,
                            bias=mask_sb[:, kc : kc + 1], scale=0.125,
                        )
                        for o in (0, 512):
                            nc.tensor.matmul(
                                st_b[:, o : o + 512],
                                kt_sb[:, hb, kc * 128 : kc * 128 + 128],
                                qt_sb[:, hp, o : o + 512],
                                start=True,
                                stop=True,
                            )
                        et_b = et_pool.tile([128, 1024], BF, tag="et")
                        nc.scalar.activation(
                            et_b[:], st_b[:], EXP,
                            bias=mask_sb[:, kc : kc + 1], scale=0.125,
                        )
                        pv_q.append((kc, et_a, et_b))
                        if kc >= 1:
                            emit_pv(*pv_q.pop(0))
                    emit_pv(*pv_q.pop(0))
                    # evacuate ctx+denominator to SBUF immediately: frees the
                    # PSUM slots for the next pair.  Head a evacuates on DVE,
                    # head b on ScalarE (activation Copy; GpSimd can't read
                    # PSUM per the BIR verifier), so the two copies run in
                    # parallel and neither queues behind the previous pair's
                    # normalize chain.
                    ctxu = norm_pool.tile([65, 2048], F32, tag="cu")
                    nc.vector.tensor_copy(ctxu[:, 0:1024], ctx_a[0:65, :])
                    nc.scalar.activation(
                        ctxu[:, 1024:2048], ctx_b[0:65, :],
                        mybir.ActivationFunctionType.Copy,
                    )
                    emit_normalize(hp, ctxu, tail=(hp == NH // 2 - 1))

                # ---- output projection: out[q, o] = ctx Wo^T + bo ----
                # Emitted split: chunks 0..4 of up to four lc groups stream
                # first (psA+psB pools, 4 open accumulation groups), the
                # chunk-5 matmuls (gated on the last head pair's normalize
                # chain) trail behind — so the PE keeps streaming while the
                # final normalize's DMA ladder completes.
                ps_of = {}

                def op_partial(lc):
                    pool = psA if (lc % 2 == 0) else psB
                    ps = pool.tile([128, 1024], F32, tag=pool is psA and "psA" or "psB")
                    ps_of[lc] = ps
                    for off, width in ((0, 512), (512, 256)):
                        for c in range(NC - 1):
                            nc.tensor.matmul(
                                ps[:, off : off + width],
                                ctxt_sb[:, c, lc * 128 : lc * 128 + 128],
                                wo_sb[:, c, off : off + width],
                                start=(c == 0),
                                stop=False,
                            )

                def op_finish(lc):
                    ps = ps_of.pop(lc)
                    c = NC - 1
                    for off, width in ((0, 512), (512, 256)):
                        nc.tensor.matmul(
                            ps[:, off : off + width],
                            ctxt_sb[:, c, lc * 128 : lc * 128 + 128],
                            wo_sb[:, c, off : off + width],
                            start=False,
                            stop=True,
                        )
                    o_sb = out_pool.tile([128, H], F32, tag="outp")
                    nc.vector.tensor_tensor(
                        o_sb[:], ps[:, 0:H], bo_sb[:], mybir.AluOpType.add
                    )
                    nc.sync.dma_start(out_e[lc * 128 : lc * 128 + 128, :], o_sb[:])

                for lc in range(4):
                    op_partial(lc)
                for lc in range(4, LC):
                    op_finish(lc - 4)
                    op_partial(lc)
                for lc in range(LC - 4, LC):
                    op_finish(lc)

    nc.finalize()
    nc.m = get_hw_module(nc.m)
    return nc


_NC_CACHE = {}


def _get_nc(compute_rounded: bool = True):
    if compute_rounded not in _NC_CACHE:
        _NC_CACHE[compute_rounded] = build_bass(compute_rounded)
    return _NC_CACHE[compute_rounded]


def make_in_maps(inputs):
    f = lambda a: np.ascontiguousarray(np.asarray(a, dtype=np.float32))  # noqa: E731
    fb = lambda a: np.ascontiguousarray(  # noqa: E731
        np.asarray(a, dtype=np.float32).astype(ml_dtypes.bfloat16)
    )
    hs = f(inputs["hidden_states"])
    mask = f(inputs["attention_mask"]).reshape(B, L)
    shared = {
        "wqt": fb(np.asarray(inputs["Wq"]).T),
        "wkt": fb(np.asarray(inputs["Wk"]).T),
        "wvt": fb(np.asarray(inputs["Wv"]).T),
        "wot": fb(np.asarray(inputs["Wo"]).T),
        "bq": f(inputs["bq"]),
        "bk": f(inputs["bk"]),
        "bv": f(inputs["bv"]),
        "bo": f(inputs["bo"]),
    }
    return [
        {"xt": fb(hs[b].T), "mask": mask[b], **shared}
        for b in range(B)
    ]


def run_spmd(inputs, trace=False, compute_rounded=True):
    nc = _get_nc(compute_rounded)
    res = run_bass_kernel_spmd(nc, make_in_maps(inputs), list(range(B)), trace=trace)
    out = np.stack([res.results[b]["out"] for b in range(B)]).astype(np.float32)
    return out, res


def kernel(**inputs) -> np.ndarray:
    out, _ = run_spmd(inputs, trace=False)
    return out


# revision 17
# speedup vs baseline: 1.5916x; 1.0285x over previous
"""BERT self-attention on 8 Trainium2 NeuronCores.

Sharding: data-parallel over batch (B=8 -> one batch element per core).
Each core computes full self-attention for its batch element:
  Q/K/V projections, per-head softmax(Q K^T / 8 + mask) V, output proj.

Layout strategy (per core):
  - Host passes xt = x.T [768,1024] and W.T [768,768] in bf16 so every
    matmul contracts over the partition axis and streams at 1 cyc/row.
  - QT,KT [d, L] and V [L, d] are produced directly by the projections.
  - Attention runs transposed: ST[k,q] = K Q^T per head, so softmax's
    reduction axis (k) lands on partitions: exp via ScalarE with the
    attention mask as per-partition bias (no max subtraction: scores are
    ~N(0,1), |s|<~7, exp is safe in fp32); the denominator comes from a
    ones column appended to V (out row 64); P^T V accumulates ctx^T
    [d, q] which feeds the output projection as lhsT directly.
  - All matmul inputs are bf16 (fp32 PSUM accumulation); measured rel
    err vs the fp32 reference is ~7e-3.
  - The PE does only the structural matmuls: QKV/out biases are folded
    into the PSUM->SBUF evacuation ops against DMA-broadcast bias rows,
    and the softmax 1/denom broadcast is computed entirely off the PE:
    the two denominator rows are DMA-packed across 128 partitions
    ([128,16]), reciprocated in one cheap DVE op, DMA-unpacked, and
    partition-broadcast on GpSimd.  This keeps ScalarE exp-only, which
    is what paces the attention inner loop.
  - Startup: weight chunks load on the Sync HWDGE ring while xt chunks
    load on the Scalar HWDGE ring, halving time-to-first-matmul.
"""

import numpy as np
import ml_dtypes

import concourse.bass as bass  # noqa: F401
import concourse.mybir as mybir
import concourse.tile as tile
from concourse import bacc
from concourse.bass_interp import get_hw_module
from concourse.bass_utils import run_bass_kernel_spmd

B, L, H = 8, 1024, 768
NH, HD = 12, 64
NC = H // 128          # 6 chunks of hidden dim
LC = L // 128          # 8 chunks of sequence dim
F32 = mybir.dt.float32
BF = mybir.dt.bfloat16
EXP = mybir.ActivationFunctionType.Exp


def build_bass(compute_rounded: bool = True):
    del compute_rounded  # single all-bf16 variant

    nc = bacc.Bacc("TRN2", debug=False, num_devices=8)

    xt_e = nc.declare_dram_parameter("xt", [H, L], BF, isOutput=False)
    wqt_e = nc.declare_dram_parameter("wqt", [H, H], BF, isOutput=False)
    wkt_e = nc.declare_dram_parameter("wkt", [H, H], BF, isOutput=False)
    wvt_e = nc.declare_dram_parameter("wvt", [H, H], BF, isOutput=False)
    wot_e = nc.declare_dram_parameter("wot", [H, H], BF, isOutput=False)
    bq_e = nc.declare_dram_parameter("bq", [H], F32, isOutput=False)
    bk_e = nc.declare_dram_parameter("bk", [H], F32, isOutput=False)
    bv_e = nc.declare_dram_parameter("bv", [H], F32, isOutput=False)
    bo_e = nc.declare_dram_parameter("bo", [H], F32, isOutput=False)
    mask_e = nc.declare_dram_parameter("mask", [L], F32, isOutput=False)
    out_e = nc.declare_dram_parameter("out", [L, H], F32, isOutput=True)
    # DRAM scratch for the softmax reciprocal broadcast roundtrip
    rscr_e = nc.dram_tensor("rscr", [2048], F32)

    with tile.TileContext(nc) as tc:
        with (
            tc.tile_pool(name="small", bufs=1) as small,
            tc.tile_pool(name="acts", bufs=1) as acts,
            tc.tile_pool(name="outp", bufs=4) as out_pool,
            tc.tile_pool(name="psA", bufs=2, space="PSUM") as psA,
            tc.tile_pool(name="psB", bufs=2, space="PSUM") as psB,
        ):
            # ---- constants / small tensors (DMAs issued after wv/xt: the
            # V-proj's first matmul gates on wv/xt chunk 0, these don't) ----
            mask_sb = small.tile([128, LC], F32)
            bq_sb = small.tile([128, NC], F32)
            bk_sb = small.tile([128, NC], F32)
            bv_sb = small.tile([128, H], F32)
            bo_sb = small.tile([128, H], F32)

            def load_smalls():
                nc.sync.dma_start(
                    mask_sb[:], mask_e[:].rearrange("(c p) -> p c", p=128)
                )
                nc.sync.dma_start(bq_sb[:], bq_e[:].rearrange("(c p) -> p c", p=128))
                nc.sync.dma_start(bk_sb[:], bk_e[:].rearrange("(c p) -> p c", p=128))
                # bias rows replicated across all partitions (free-dim biases)
                nc.sync.dma_start(bv_sb[:], bv_e[None, :].to_broadcast((128, H)))
                nc.sync.dma_start(bo_sb[:], bo_e[None, :].to_broadcast((128, H)))

            qt_sb = acts.tile([128, NC, L], BF)
            kt_sb = acts.tile([128, NH, L], BF)  # per-head K^T, other 64 rows zero
            nc.gpsimd.memset(kt_sb[:], 0.0)
            v_sb = acts.tile([128, LC, NH, HD + 1], BF)  # [..., 64] = ones col
            ctxt_sb = acts.tile([128, NC, L], BF)
            nc.vector.memset(v_sb[:, :, :, HD], 1.0)

            # =========== projection phase (xt + wv/wk/wq scoped) ===========
            with (
                tc.tile_pool(name="xt", bufs=1) as xt_pool,
                tc.tile_pool(name="w1", bufs=2) as w1,
            ):
                # weights ride the Sync HWDGE ring, xt the Scalar ring, so
                # the V-proj's first matmul only waits ~one chunk per ring
                xt_sb = xt_pool.tile([128, NC, L], BF)
                wv_sb = w1.tile([128, NC, H], BF, tag="w")
                for c in range(NC):
                    eng_w = nc.sync if c % 2 == 0 else nc.scalar
                    eng_x = nc.scalar if c % 2 == 0 else nc.sync
                    eng_w.dma_start(
                        wv_sb[:, c, :],
                        wvt_e[:].rearrange("(c p) d -> p c d", p=128)[:, c, :],
                    )
                    eng_x.dma_start(
                        xt_sb[:, c, :],
                        xt_e[:].rearrange("(c p) q -> p c q", p=128)[:, c, :],
                    )
                load_smalls()
                for lc in range(LC):
                    pool = psA if lc % 2 == 0 else psB
                    ps = pool.tile([128, 1024], F32, tag="psA" if lc % 2 == 0 else "psB")
                    for off, width in ((0, 512), (512, 256)):
                        for kc in range(NC):
                            nc.tensor.matmul(
                                ps[:, off : off + width],
                                xt_sb[:, kc, lc * 128 : lc * 128 + 128],
                                wv_sb[:, kc, off : off + width],
                                start=(kc == 0),
                                stop=(kc == NC - 1),
                            )
                    # evacuate + bv add (bias varies along free dim)
                    nc.vector.tensor_tensor(
                        v_sb[:, lc, :, 0:HD],
                        ps[:, 0:H].rearrange("p (h d) -> p h d", d=HD),
                        bv_sb[:].rearrange("p (h d) -> p h d", d=HD),
                        mybir.AluOpType.add,
                    )

                # ---- K^T then Q^T projections: out[d, q] = W x^T + b
                for w_e, b_sb, dst in ((wkt_e, bk_sb, kt_sb), (wqt_e, bq_sb, qt_sb)):
                    w_sb = w1.tile([128, NC, H], BF, tag="w")
                    nc.sync.dma_start(
                        w_sb[:], w_e[:].rearrange("(c p) d -> p c d", p=128)
                    )
                    for dc in range(NC):
                        pool = psA if dc % 2 == 0 else psB
                        ps = pool.tile(
                            [128, 1024], F32, tag="psA" if dc % 2 == 0 else "psB"
                        )
                        for qh in range(2):
                            o = qh * 512
                            for kc in range(NC):
                                nc.tensor.matmul(
                                    ps[:, o : o + 512],
                                    w_sb[:, kc, dc * 128 : dc * 128 + 128],
                                    xt_sb[:, kc, o : o + 512],
                                    start=(kc == 0),
                                    stop=(kc == NC - 1),
                                )
                        if dst is qt_sb:
                            nc.vector.tensor_scalar_add(
                                dst[:, dc, :], ps[:, :], b_sb[:, dc : dc + 1]
                            )
                        else:
                            nc.vector.tensor_scalar_add(
                                kt_sb[0:64, 2 * dc, :], ps[0:64, :],
                                b_sb[0:64, dc : dc + 1],
                            )
                            nc.vector.tensor_scalar_add(
                                kt_sb[64:128, 2 * dc + 1, :], ps[64:128, :],
                                b_sb[64:128, dc : dc + 1],
                            )

            # =========== attention + output projection ===========
            with (
                tc.tile_pool(name="w2", bufs=1) as w2,
                tc.tile_pool(name="et", bufs=6) as et_pool,
                tc.tile_pool(name="norm", bufs=2) as norm_pool,
            ):
                wo_sb = w2.tile([128, NC, H], BF)
                nc.sync.dma_start(
                    wo_sb[:], wot_e[:].rearrange("(c p) d -> p c d", p=128)
                )

                def emit_normalize(hp, ctxu, tail=False):
                    # softmax 1/denom, entirely off the PE/ACT engines:
                    # pack both heads' denominator rows across 128 partitions
                    # (q = p*8+i per head), one cheap DVE reciprocal, then a
                    # DRAM roundtrip to replicate the reciprocals to 64 rows
                    # (engines can't broadcast across partitions; DMA can't
                    # use a 0-stride SBUF source, but a DRAM source works).
                    dpk = norm_pool.tile([128, 16], F32, tag="dpk")
                    nc.sync.dma_start(
                        dpk[:, 0:8],
                        ctxu[64:65, 0:1024].rearrange("o (p i) -> o p i", p=128),
                    )
                    nc.sync.dma_start(
                        dpk[:, 8:16],
                        ctxu[64:65, 1024:2048].rearrange("o (p i) -> o p i", p=128),
                    )
                    rpk = norm_pool.tile([128, 16], F32, tag="rpk")
                    nc.vector.reciprocal(rpk[:], dpk[:])
                    # scatter-write the 8KB of reciprocals into q-order in
                    # DRAM, so the two 256KB broadcast reads below stream
                    # fully contiguous 4KB rows (a 32B-granule broadcast
                    # read measured ~5us; this shape is ~1us)
                    nc.sync.dma_start(
                        rscr_e[:].rearrange("(h p i) -> p h i", h=2, p=128),
                        rpk[:].rearrange("p (h i) -> p h i", h=2),
                    )
                    bc = norm_pool.tile([64, 2048], F32, tag="bc")
                    rscr_hq = rscr_e[:].rearrange("(h q) -> h q", h=2)
                    # at the tail ScalarE is done with exps, so its HWDGE
                    # ring can carry one broadcast in parallel
                    eng_b = nc.scalar if tail else nc.sync
                    nc.sync.dma_start(
                        bc[:, 0:1024], rscr_hq[0:1, :].to_broadcast((64, 1024))
                    )
                    eng_b.dma_start(
                        bc[:, 1024:2048], rscr_hq[1:2, :].to_broadcast((64, 1024))
                    )
                    nc.vector.tensor_tensor(
                        ctxt_sb[0:64, hp, :], ctxu[0:64, 0:1024], bc[:, 0:1024],
                        mybir.AluOpType.mult,
                    )
                    tmp_o = norm_pool.tile([64, 1024], BF, tag="tmp")
                    nc.vector.tensor_tensor(
                        tmp_o[:], ctxu[0:64, 1024:2048], bc[:, 1024:2048],
                        mybir.AluOpType.mult,
                    )
                    # lift odd head to partitions 64:128 (DMA can cross
                    # partitions; DVE cannot)
                    nc.sync.dma_start(ctxt_sb[64:128, hp, :], tmp_o[:])

                for hp in range(NH // 2):
                    ha, hb = 2 * hp, 2 * hp + 1
                    # per head: rows 0:64 = ctx^T, row 64 = softmax denominator
                    ctx_a = psB.tile([128, 1024], F32, tag="psB")
                    ctx_b = psB.tile([128, 1024], F32, tag="psB")

                    def emit_pv(kc, et_a, et_b, ctx_a=ctx_a, ctx_b=ctx_b,
                                ha=ha, hb=hb):
                        first, last = kc == 0, kc == LC - 1
                        for qh in range(2):
                            o = qh * 512
                            # ctx^T[d, q] += V^T P^T ; row 64 = denominator
                            nc.tensor.matmul(
                                ctx_a[0 : HD + 1, o : o + 512],
                                v_sb[:, kc, ha, :],
                                et_a[:, o : o + 512],
                                start=first,
                                stop=last,
                            )
                            nc.tensor.matmul(
                                ctx_b[0 : HD + 1, o : o + 512],
                                v_sb[:, kc, hb, :],
                                et_b[:, o : o + 512],
                                start=first,
                                stop=last,
                            )

                    pv_q = []  # software pipeline: PV(kc-1) after ST(kc)
                    for kc in range(LC):
                        # per-head ST then its exp immediately, so ScalarE
                        # starts head a's exp while the PE streams head b
                        st_a = psA.tile([128, 1024], F32, tag="psA")
                        st_b = psA.tile([128, 1024], F32, tag="psA")
                        for o in (0, 512):
                            nc.tensor.matmul(
                                st_a[:, o : o + 512],
                                kt_sb[:, ha, kc * 128 : kc * 128 + 128],
                                qt_sb[:, hp, o : o + 512],
                                start=True,
                                stop=True,
                            )
                        et_a = et_pool.tile([128, 1024], BF, tag="et")
                        nc.scalar.activation(
                            et_a[:], st_a[:], EXP,
                            bias=mask_sb[:, kc : kc + 1], scale=0.125,
                        )
                        for o in (0, 512):
                            nc.tensor.matmul(
                                st_b[:, o : o + 512],
                                kt_sb[:, hb, kc * 128 : kc * 128 + 128],
                                qt_sb[:, hp, o : o + 512],
                                start=True,
                                stop=True,
                            )
                        et_b = et_pool.tile([128, 1024], BF, tag="et")
                        nc.scalar.activation(
                            et_b[:], st_b[:], EXP,
                            bias=mask_sb[:, kc : kc + 1], scale=0.125,
                        )
                        pv_q.append((kc, et_a, et_b))
                        if kc >= 1:
                            emit_pv(*pv_q.pop(0))
                    emit_pv(*pv_q.pop(0))
                    # evacuate ctx+denominator to SBUF immediately: frees the
                    # PSUM slots for the next pair.  Both heads evacuate on
                    # DVE back-to-back (keeping ScalarE exp-only: an ACT
                    # copy displaces the next pair's first exp by ~1us).
                    ctxu = norm_pool.tile([65, 2048], F32, tag="cu")
                    nc.vector.tensor_copy(ctxu[:, 0:1024], ctx_a[0:65, :])
                    nc.vector.tensor_copy(ctxu[:, 1024:2048], ctx_b[0:65, :])
                    emit_normalize(hp, ctxu, tail=(hp == NH // 2 - 1))

                # ---- output projection: out[q, o] = ctx Wo^T + bo ----
                # Emitted split: chunks 0..4 of up to four lc groups stream
                # first (psA+psB pools, 4 open accumulation groups), the
                # chunk-5 matmuls (gated on the last head pair's normalize
                # chain) trail behind — so the PE keeps streaming while the
                # final normalize's DMA ladder completes.
                ps_of = {}

                def op_partial(lc):
                    pool = psA if (lc % 2 == 0) else psB
                    ps = pool.tile([128, 1024], F32, tag=pool is psA and "psA" or "psB")
                    ps_of[lc] = ps
                    for off, width in ((0, 512), (512, 256)):
                        for c in range(NC - 1):
                            nc.tensor.matmul(
                                ps[:, off : off + width],
                                ctxt_sb[:, c, lc * 128 : lc * 128 + 128],
                                wo_sb[:, c, off : off + width],
                                start=(c == 0),
                                stop=False,
                            )

                def op_finish(lc):
                    ps = ps_of.pop(lc)
                    c = NC - 1
                    for off, width in ((0, 512), (512, 256)):
                        nc.tensor.matmul(
                            ps[:, off : off + width],
                            ctxt_sb[:, c, lc * 128 : lc * 128 + 128],
                            wo_sb[:, c, off : off + width],
                            start=False,
                            stop=True,
                        )
                    o_sb = out_pool.tile([128, H], F32, tag="outp")
                    nc.vector.tensor_tensor(
                        o_sb[:], ps[:, 0:H], bo_sb[:], mybir.AluOpType.add
                    )
                    # stores alternate rings (ScalarE is idle by now)
                    eng_s = nc.sync if lc % 2 == 0 else nc.scalar
                    eng_s.dma_start(out_e[lc * 128 : lc * 128 + 128, :], o_sb[:])

                for lc in range(4):
                    op_partial(lc)
                for lc in range(4, LC):
                    op_finish(lc - 4)
                    op_partial(lc)
                for lc in range(LC - 4, LC):
                    op_finish(lc)

    nc.finalize()
    nc.m = get_hw_module(nc.m)
    return nc


_NC_CACHE = {}


def _get_nc(compute_rounded: bool = True):
    if compute_rounded not in _NC_CACHE:
        _NC_CACHE[compute_rounded] = build_bass(compute_rounded)
    return _NC_CACHE[compute_rounded]


def make_in_maps(inputs):
    f = lambda a: np.ascontiguousarray(np.asarray(a, dtype=np.float32))  # noqa: E731
    fb = lambda a: np.ascontiguousarray(  # noqa: E731
        np.asarray(a, dtype=np.float32).astype(ml_dtypes.bfloat16)
    )
    hs = f(inputs["hidden_states"])
    mask = f(inputs["attention_mask"]).reshape(B, L)
    shared = {
        "wqt": fb(np.asarray(inputs["Wq"]).T),
        "wkt": fb(np.asarray(inputs["Wk"]).T),
        "wvt": fb(np.asarray(inputs["Wv"]).T),
        "wot": fb(np.asarray(inputs["Wo"]).T),
        "bq": f(inputs["bq"]),
        "bk": f(inputs["bk"]),
        "bv": f(inputs["bv"]),
        "bo": f(inputs["bo"]),
    }
    return [
        {"xt": fb(hs[b].T), "mask": mask[b], **shared}
        for b in range(B)
    ]


def run_spmd(inputs, trace=False, compute_rounded=True):
    nc = _get_nc(compute_rounded)
    res = run_bass_kernel_spmd(nc, make_in_maps(inputs), list(range(B)), trace=trace)
    out = np.stack([res.results[b]["out"] for b in range(B)]).astype(np.float32)
    return out, res


def kernel(**inputs) -> np.ndarray:
    out, _ = run_spmd(inputs, trace=False)
    return out
